# revision 13
# baseline (speedup 1.0000x reference)
"""Multi-head attention (ReLU-gated projections) on 8 Trainium2 NeuronCores.

Problem (hardcoded): B=4, S=1024, H=1024, NH=16, DH=64.
  qp = relu(q @ Wq.T + bq); kp, vp likewise
  alpha = softmax(qh @ kh.T / sqrt(DH)) * mask[q]
  out = (alpha @ vh).reshape(B,S,H) + query

Sharding: 8 cores = 4 batches x 2 head-groups (8 heads / 512 hidden cols each).

Per-core device kernel (all in transposed "hidden-on-partitions" layout):
  stage 1: qpT[o,s], kpT[o,s] (transposed) and vp[s,o] (normal) projections
           with fused bias+relu. Contraction over h via PE; inputs fed
           host-pre-transposed (xT = x.T per batch).
  stage 2: per head: alphaT[k,q] = khT.T @ qhT (K=64); P=exp(alpha/8) on ACT
           (no max subtraction needed: alpha/8 <= ~5); AV via PE with a ones
           column appended to v so row 64 of the output accumulates
           sumexp[q] for free.  Output: unnormalized hidT (64,S) + sumexp (S)
           per head; host divides, applies mask, adds residual.
"""
import sys

sys.path.insert(0, "/opt/trn_rl_repo")

import os
import numpy as np
import ml_dtypes

import concourse.bass as bass
import concourse.tile as tile
from concourse import bacc, mybir
from concourse import bass_utils

B, S, H = 4, 1024, 1024
NH, DH = 16, 64
NCORES = 8
GROUPS = 2          # head-groups (tensor-parallel dim)
HL = NH // GROUPS   # heads per core = 8
GH = H // GROUPS    # hidden cols per core = 512
KT = H // 128       # contraction k-tiles = 8
OT = GH // 128      # output o-tiles per core = 4
SCALE = 1.0 / float(np.sqrt(DH))

# matmul precision mode: "f32" (exact, 4 cyc/row), "f32r" (TF32-ish, 1 cyc/row),
# "bf16" (1 cyc/row, smallest footprint)
MODE = os.environ.get("BASS_MM_DT", "f32r")
ALPHA_ILV = os.environ.get("BASS_ALPHA_ILV", "1") == "1"

F32 = mybir.dt.float32
F32R = mybir.dt.float32r
BF16 = mybir.dt.bfloat16


def _cfg(mode):
    if mode == "bf16":
        return dict(np_dt=ml_dtypes.bfloat16, io_dt=BF16, st_dt=BF16,
                    cast=False, pt_bufs=36, hid_bufs=3, x_bufs=16,
                    shift_alphas=True)
    if mode == "f32r":
        # float32r end-to-end: walrus requires f32r matmul inputs to be
        # *produced* as f32r (DMA loads + DVE/ACT evacuations), not bitcast.
        return dict(np_dt=np.float32, io_dt=F32R, st_dt=F32R,
                    cast=False, pt_bufs=9, hid_bufs=2, x_bufs=8,
                    shift_alphas=False)
    return dict(np_dt=np.float32, io_dt=F32, st_dt=F32,
                cast=False, pt_bufs=9, hid_bufs=2, x_bufs=8,
                shift_alphas=False)


def _mm(ap, cast):
    return ap.bitcast(F32R) if cast else ap


def build(mode):
    cfg = _cfg(mode)
    io_dt, st_dt, cast = cfg["io_dt"], cfg["st_dt"], cfg["cast"]
    nc = bacc.Bacc("TRN2", target_bir_lowering=False, debug=False,
                   num_devices=NCORES)

    xq_d = nc.dram_tensor("xq", [H, S], io_dt, kind="ExternalInput").ap()
    xk_d = nc.dram_tensor("xk", [H, S], io_dt, kind="ExternalInput").ap()
    xv_d = nc.dram_tensor("xv", [H, S], io_dt, kind="ExternalInput").ap()
    wq_d = nc.dram_tensor("wq", [H, GH], io_dt, kind="ExternalInput").ap()
    wk_d = nc.dram_tensor("wk", [H, GH], io_dt, kind="ExternalInput").ap()
    wv_d = nc.dram_tensor("wv", [H, GH], io_dt, kind="ExternalInput").ap()
    bqk_d = nc.dram_tensor("bqk", [128, 2 * OT], F32, kind="ExternalInput").ap()
    bv_d = nc.dram_tensor("bv", [1, GH], io_dt, kind="ExternalInput").ap()
    ones_d = nc.dram_tensor("onesd", [128, 128], io_dt,
                            kind="ExternalInput").ap()
    hid_d = nc.dram_tensor("hid", [HL * (DH + 1), S], F32,
                           kind="ExternalOutput").ap()

    with tile.TileContext(nc) as tc:
        with tc.tile_pool(name="sb", bufs=1) as sb, \
             tc.tile_pool(name="ps", bufs=1, space="PSUM") as ps:

            full_x = mode == "bf16"   # x resident for full S vs per-chunk

            # ---- persistent tiles; one big DMA per tensor (>=1MB, descriptor
            #      runs of 1-2KB/partition), spread across the three DGE rings
            #      (sync / scalar / gpsimd) so loads overlap ----
            wq_t = sb.tile([128, KT * GH], io_dt, tag="wq", name="wq")
            wk_t = sb.tile([128, KT * GH], io_dt, tag="wk", name="wk")
            wv_t = sb.tile([128, KT * GH], io_dt, tag="wv", name="wv")
            qp_t = [sb.tile([128, S], st_dt, tag=f"qp{t}", name=f"qp{t}")
                    for t in range(OT)]
            kp_t = [sb.tile([128, S], st_dt, tag=f"kp{t}", name=f"kp{t}")
                    for t in range(OT)]
            # v laid out [k-tile x head x (64 v cols + ones col)]
            VW = HL * (DH + 1)
            vp_t = sb.tile([128, KT * VW], st_dt, tag="vp", name="vp")
            bqk_t = sb.tile([128, 2 * OT], F32, tag="bqk", name="bqk")
            bv_t = sb.tile([1, GH], io_dt, tag="bv", name="bv")
            ones_t = sb.tile([1, 128], io_dt, tag="ones", name="ones")

            def w_ld(w_t, w_d, eng):
                # SBUF [128, k*GH + o]  <-  DRAM [(k p) o]
                eng.dma_start(w_t[:].rearrange("p (k o) -> p k o", o=GH),
                              w_d.rearrange("(k p) o -> p k o", p=128))

            x_t = {}

            def x_ld(which, x_d, eng, sc):
                # one chunk (512 s-cols) of one input, laid [128, k*512+s]
                tag = f"x{which}{sc}" if full_x else f"x{which}"
                t = sb.tile([128, KT * 512], io_dt, tag=tag,
                            name=f"x{which}_{sc}")
                eng.dma_start(
                    t[:].rearrange("p (k s) -> p k s", s=512),
                    x_d.rearrange("(k p) s -> p k s",
                                  p=128)[:, :, sc * 512:(sc + 1) * 512])
                x_t[(which, sc)] = t

            def xap(which, sc, k):
                return x_t[(which, sc)][:, k * 512:(k + 1) * 512]

            nc.sync.dma_start(bv_t[:], bv_d)
            nc.sync.dma_start(ones_t[:], ones_d[0:1, :])
            nc.sync.dma_start(bqk_t[:], bqk_d)
            ones64_t = sb.tile([128, KT * HL], io_dt, tag="ones64",
                               name="ones64")
            nc.sync.dma_start(ones64_t[:], ones_d[:, 0:KT * HL])
            x_ld("q", xq_d, nc.scalar, 0)
            w_ld(wq_t, wq_d, nc.sync)
            x_ld("k", xk_d, nc.sync, 0)
            w_ld(wk_t, wk_d, nc.scalar)
            x_ld("v", xv_d, nc.scalar, 0)
            w_ld(wv_t, wv_d, nc.sync)

            # HAM warmup: ~7us of tiny matmuls on early-arriving const tiles
            # so the real matmuls start at 2.4GHz instead of 1.2
            warm = ps.tile([128, 512], F32, tag="av", bufs=2, name="warm")
            for i in range(20):
                nc.tensor.matmul(warm[:], _mm(ones_t[:], cast),
                                 _mm(bv_t[:], cast), start=True, stop=True)
            if full_x:
                x_ld("q", xq_d, nc.sync, 1)
                x_ld("k", xk_d, nc.scalar, 1)
                x_ld("v", xv_d, nc.scalar, 1)
            v4 = vp_t[:].rearrange("p (k n c) -> p k n c", n=HL, c=DH + 1)
            nc.vector.tensor_copy(
                v4[:, :, :, DH:DH + 1],
                ones64_t[:].rearrange("p (k n one) -> p k n one", n=HL, one=1))

            def proj_qk(sc, ot):
                """one o-tile, one s-chunk of the transposed q/k projections"""
                for which, w_t, xw, o_t in (("q", wq_t, "q", qp_t),
                                            ("k", wk_t, "k", kp_t)):
                    pp = ps.tile([128, 1024], F32, tag="alpha", bufs=3,
                                 name=f"pp{which}_{sc}_{ot}")
                    for k in range(KT):
                        nc.tensor.matmul(
                            pp[:, 0:512],
                            _mm(w_t[:, k * GH + ot * 128:
                                    k * GH + (ot + 1) * 128], cast),
                            _mm(xap(xw, sc, k), cast),
                            start=(k == 0), stop=(k == KT - 1))
                    wi = 0 if which == "q" else 1
                    bias = bqk_t[:, wi * OT + ot:wi * OT + ot + 1]
                    nc.vector.tensor_scalar(
                        o_t[ot][:, sc * 512:(sc + 1) * 512], pp[:, 0:512],
                        bias, 0.0, mybir.AluOpType.add, mybir.AluOpType.max)

            def proj_v(sc, j):
                """one s-tile (128 rows of vp) within chunk sc"""
                st = sc * 4 + j
                pp = ps.tile([128, 1024], F32, tag="alpha", bufs=3,
                             name=f"ppv_{st}")
                nc.tensor.matmul(pp[:, 0:512], _mm(ones_t[:], cast),
                                 _mm(bv_t[:], cast), start=True, stop=False)
                for k in range(KT):
                    nc.tensor.matmul(
                        pp[:, 0:512],
                        _mm(xap("v", sc, k)[:, j * 128:(j + 1) * 128], cast),
                        _mm(wv_t[:, k * GH:(k + 1) * GH], cast),
                        start=False, stop=(k == KT - 1))
                v3 = vp_t[:, st * VW:(st + 1) * VW].rearrange(
                    "p (n c) -> p n c", c=DH + 1)
                p3 = pp[:, 0:512].rearrange("p (n c) -> p n c", c=DH)
                nc.vector.tensor_scalar(
                    v3[:, :, 0:DH], p3, 0.0, None, mybir.AluOpType.max)

            pt_all = {}

            def alphas(n0):
                """alpha + exp for head pair (n0, n0+1); the two heads live on
                disjoint 64-partition halves of o-tile n0//2, so adjacent
                matmuls target disjoint PE row-groups and overlap."""
                t = n0 // 2
                pts0, pts1 = [], []
                for k in range(KT):
                    apts = []
                    for h in range(2):
                        apt = ps.tile([128, 1024], F32, tag="alpha", bufs=3,
                                      name=f"alp_{n0 + h}_{k}")
                        apts.append(apt)
                    order = ([(qc, h) for qc in range(2) for h in range(2)]
                             if ALPHA_ILV else
                             [(qc, h) for h in range(2) for qc in range(2)])
                    for qc, h in order:
                        off = h * 64
                        nc.tensor.matmul(
                            apts[h][:, qc * 512:(qc + 1) * 512],
                            _mm(kp_t[t][off:off + 64,
                                        k * 128:(k + 1) * 128], cast),
                            _mm(qp_t[t][off:off + 64,
                                        qc * 512:(qc + 1) * 512], cast),
                            start=True, stop=True)
                    for h, pts in ((0, pts0), (1, pts1)):
                        pt = sb.tile([128, 1024], st_dt, tag="pt",
                                     bufs=cfg["pt_bufs"], name=f"pt_{n0 + h}_{k}")
                        nc.scalar.activation(pt[:], apts[h][:],
                                             mybir.ActivationFunctionType.Exp,
                                             scale=SCALE)
                        pts.append(pt)
                pt_all[n0] = pts0
                pt_all[n0 + 1] = pts1

            def head_seq(n):
                """unpaired alpha+exp then AV for one head (low pt_bufs modes)"""
                t, off = n // 2, (n % 2) * 64
                pts = []
                for k in range(KT):
                    apt = ps.tile([128, 1024], F32, tag="alpha", bufs=3,
                                  name=f"alp_{n}_{k}")
                    for qc in range(2):
                        nc.tensor.matmul(
                            apt[:, qc * 512:(qc + 1) * 512],
                            _mm(kp_t[t][off:off + 64,
                                        k * 128:(k + 1) * 128], cast),
                            _mm(qp_t[t][off:off + 64,
                                        qc * 512:(qc + 1) * 512], cast),
                            start=True, stop=True)
                    pt = sb.tile([128, 1024], st_dt, tag="pt",
                                 bufs=cfg["pt_bufs"], name=f"pt_{n}_{k}")
                    nc.scalar.activation(pt[:], apt[:],
                                         mybir.ActivationFunctionType.Exp,
                                         scale=SCALE)
                    pts.append(pt)
                pt_all[n] = pts

            def avs(n):
                pts = pt_all.pop(n)
                hid_t = sb.tile([DH + 1, S], F32, tag="hid",
                                bufs=cfg["hid_bufs"], name=f"hid_{n}")
                for qc in range(2):
                    av = ps.tile([DH + 1, 512], F32, tag="av", bufs=2,
                                 name=f"av_{n}_{qc}")
                    for k in range(KT):
                        nc.tensor.matmul(
                            av[:],
                            _mm(vp_t[:, k * VW + n * (DH + 1):
                                     k * VW + (n + 1) * (DH + 1)], cast),
                            _mm(pts[k][:, qc * 512:(qc + 1) * 512], cast),
                            start=(k == 0), stop=(k == KT - 1))
                    nc.vector.tensor_copy(
                        hid_t[:, qc * 512:(qc + 1) * 512], av[:])
                    nc.sync.dma_start(
                        hid_d[n * (DH + 1):(n + 1) * (DH + 1),
                              qc * 512:(qc + 1) * 512],
                        hid_t[:, qc * 512:(qc + 1) * 512])

            # ---- emission schedule ----
            if cfg["shift_alphas"]:
                proj_qk(0, 0)
                proj_qk(1, 0)
            else:
                for ot in range(OT):
                    proj_qk(0, ot)
                for j in range(4):
                    proj_v(0, j)
            if not full_x:
                x_ld("q", xq_d, nc.sync, 1)
                x_ld("k", xk_d, nc.scalar, 1)
                x_ld("v", xv_d, nc.scalar, 1)
            else:
                pass  # chunk-1 loads were emitted upfront
            if cfg["shift_alphas"]:
                alphas(0)
                proj_qk(0, 1)
                proj_qk(1, 1)
                alphas(2)
                for j in range(4):
                    proj_v(0, j)
                for j in range(4):
                    proj_v(1, j)
                avs(0)
                avs(1)
                proj_qk(0, 2)
                proj_qk(1, 2)
                alphas(4)
                avs(2)
                avs(3)
                proj_qk(0, 3)
                proj_qk(1, 3)
                alphas(6)
                avs(4)
                avs(5)
                avs(6)
                avs(7)
            else:
                proj_qk(1, 0)
                head_seq(0)
                for j in range(4):
                    proj_v(1, j)
                head_seq(1)
                avs(0)
                avs(1)
                for ot in range(1, OT):
                    proj_qk(1, ot)
                    head_seq(2 * ot)
                    avs(2 * ot)
                    head_seq(2 * ot + 1)
                    avs(2 * ot + 1)

    nc.compile()
    return nc


_NC_CACHE = {}


def _get_nc(mode):
    if mode not in _NC_CACHE:
        _NC_CACHE[mode] = build(mode)
    return _NC_CACHE[mode]


def _prep_inputs(inputs, mode):
    cfg = _cfg(mode)
    np_dt = cfg["np_dt"]
    q = np.asarray(inputs["query"], np.float32)
    k = np.asarray(inputs["key"], np.float32)
    v = np.asarray(inputs["value"], np.float32)
    Wq = np.asarray(inputs["Wq"], np.float32)
    Wk = np.asarray(inputs["Wk"], np.float32)
    Wv = np.asarray(inputs["Wv"], np.float32)
    bq = np.asarray(inputs["bq"], np.float32)
    bk = np.asarray(inputs["bk"], np.float32)
    bv = np.asarray(inputs["bv"], np.float32)

    xq = [np.ascontiguousarray(q[b].T).astype(np_dt) for b in range(B)]
    xk = [np.ascontiguousarray(k[b].T).astype(np_dt) for b in range(B)]
    xv = [np.ascontiguousarray(v[b].T).astype(np_dt) for b in range(B)]
    in_maps = []
    for c in range(NCORES):
        b, g = c // GROUPS, c % GROUPS
        sl = slice(g * GH, (g + 1) * GH)
        bqk = np.stack([bq[sl].reshape(OT, 128).T,
                        bk[sl].reshape(OT, 128).T], 1).reshape(128, 2 * OT)
        in_maps.append({
            "xq": xq[b], "xk": xk[b], "xv": xv[b],
            "wq": np.ascontiguousarray(Wq[sl, :].T).astype(np_dt),
            "wk": np.ascontiguousarray(Wk[sl, :].T).astype(np_dt),
            "wv": np.ascontiguousarray(Wv[sl, :].T).astype(np_dt),
            "bqk": np.ascontiguousarray(bqk, dtype=np.float32),
            "bv": np.ascontiguousarray(bv[None, sl]).astype(np_dt),
            "onesd": np.ones((128, 128), np_dt),
        })
    return in_maps


def run(inputs, mode=MODE, trace=False):
    nc = _get_nc(mode)
    in_maps = _prep_inputs(inputs, mode)
    res = bass_utils.run_bass_kernel_spmd(
        nc, in_maps, core_ids=list(range(NCORES)), trace=trace)

    masks = np.asarray(inputs["masks"], np.float32)
    query = np.asarray(inputs["query"], np.float32)
    out = np.empty((B, S, H), np.float32)
    for c in range(NCORES):
        b, g = c // GROUPS, c % GROUPS
        hid = res.results[c]["hid"].reshape(HL, DH + 1, S)
        hT = hid[:, :DH, :]                      # (HL, DH, S)
        se = hid[:, DH, :]                       # (HL, S)
        blk = (hT / se[:, None, :]).transpose(2, 0, 1).reshape(S, GH)
        out[b, :, g * GH:(g + 1) * GH] = blk
    out = out * masks[:, :, None] + query
    return out, res


def kernel(**inputs) -> np.ndarray:
    out, _ = run(inputs)
    return out


# revision 14
# speedup vs baseline: 1.1361x; 1.1361x over previous
"""Multi-head attention (ReLU-gated projections) on 8 Trainium2 NeuronCores.

Problem (hardcoded): B=4, S=1024, H=1024, NH=16, DH=64.
  qp = relu(q @ Wq.T + bq); kp, vp likewise
  alpha = softmax(qh @ kh.T / sqrt(DH)) * mask[q]
  out = (alpha @ vh).reshape(B,S,H) + query

Sharding: 8 cores = 4 batches x 2 head-groups (8 heads / 512 hidden cols each).

Per-core device kernel (all in transposed "hidden-on-partitions" layout):
  stage 1: qpT[o,s], kpT[o,s] (transposed) and vp[s,o] (normal) projections
           with fused bias+relu. Contraction over h via PE; inputs fed
           host-pre-transposed (xT = x.T per batch).
  stage 2: per head: alphaT[k,q] = khT.T @ qhT (K=64); P=exp(alpha/8) on ACT
           (no max subtraction needed: alpha/8 <= ~5); AV via PE with a ones
           column appended to v so row 64 of the output accumulates
           sumexp[q] for free.  Output: unnormalized hidT (64,S) + sumexp (S)
           per head; host divides, applies mask, adds residual.
"""
import sys

sys.path.insert(0, "/opt/trn_rl_repo")

import os
import numpy as np
import ml_dtypes

import concourse.bass as bass
import concourse.tile as tile
from concourse import bacc, mybir
from concourse import bass_utils

B, S, H = 4, 1024, 1024
NH, DH = 16, 64
NCORES = 8
GROUPS = 2          # head-groups (tensor-parallel dim)
HL = NH // GROUPS   # heads per core = 8
GH = H // GROUPS    # hidden cols per core = 512
KT = H // 128       # contraction k-tiles = 8
OT = GH // 128      # output o-tiles per core = 4
SCALE = 1.0 / float(np.sqrt(DH))

# matmul precision mode: "f32" (exact, 4 cyc/row), "f32r" (TF32-ish, 1 cyc/row),
# "bf16" (1 cyc/row, smallest footprint)
MODE = os.environ.get("BASS_MM_DT", "f32r")
ALPHA_ILV = os.environ.get("BASS_ALPHA_ILV", "1") == "1"

F32 = mybir.dt.float32
F32R = mybir.dt.float32r
BF16 = mybir.dt.bfloat16


def _cfg(mode):
    if mode == "bf16":
        return dict(np_dt=ml_dtypes.bfloat16, io_dt=BF16, st_dt=BF16,
                    cast=False, pt_bufs=36, hid_bufs=3, x_bufs=16,
                    shift_alphas=True)
    if mode == "f32r":
        # float32r end-to-end: walrus requires f32r matmul inputs to be
        # *produced* as f32r (DMA loads + DVE/ACT evacuations), not bitcast.
        return dict(np_dt=np.float32, io_dt=F32R, st_dt=F32R,
                    cast=False, pt_bufs=9, hid_bufs=2, x_bufs=8,
                    shift_alphas=False)
    return dict(np_dt=np.float32, io_dt=F32, st_dt=F32,
                cast=False, pt_bufs=9, hid_bufs=2, x_bufs=8,
                shift_alphas=False)


def _mm(ap, cast):
    return ap.bitcast(F32R) if cast else ap


def build(mode):
    cfg = _cfg(mode)
    io_dt, st_dt, cast = cfg["io_dt"], cfg["st_dt"], cfg["cast"]
    nc = bacc.Bacc("TRN2", target_bir_lowering=False, debug=False,
                   num_devices=NCORES)

    xq_d = nc.dram_tensor("xq", [H, S], io_dt, kind="ExternalInput").ap()
    xk_d = nc.dram_tensor("xk", [H, S], io_dt, kind="ExternalInput").ap()
    xv_d = nc.dram_tensor("xv", [H, S], io_dt, kind="ExternalInput").ap()
    wq_d = nc.dram_tensor("wq", [H, GH], io_dt, kind="ExternalInput").ap()
    wk_d = nc.dram_tensor("wk", [H, GH], io_dt, kind="ExternalInput").ap()
    wv_d = nc.dram_tensor("wv", [H, GH], io_dt, kind="ExternalInput").ap()
    bqk_d = nc.dram_tensor("bqk", [128, 2 * OT], F32, kind="ExternalInput").ap()
    bv_d = nc.dram_tensor("bv", [1, GH], io_dt, kind="ExternalInput").ap()
    ones_d = nc.dram_tensor("onesd", [128, 128], io_dt,
                            kind="ExternalInput").ap()
    hid_d = nc.dram_tensor("hid", [HL * (DH + 1), S], F32,
                           kind="ExternalOutput").ap()

    with tile.TileContext(nc) as tc:
        with tc.tile_pool(name="sb", bufs=1) as sb, \
             tc.tile_pool(name="ps", bufs=1, space="PSUM") as ps:

            full_x = mode == "bf16"   # x resident for full S vs per-chunk

            # ---- persistent tiles; one big DMA per tensor (>=1MB, descriptor
            #      runs of 1-2KB/partition), spread across the three DGE rings
            #      (sync / scalar / gpsimd) so loads overlap ----
            wq_t = sb.tile([128, KT * GH], io_dt, tag="wq", name="wq")
            wk_t = sb.tile([128, KT * GH], io_dt, tag="wk", name="wk")
            wv_t = sb.tile([128, KT * GH], io_dt, tag="wv", name="wv")
            qp_t = [sb.tile([128, S], st_dt, tag=f"qp{t}", name=f"qp{t}")
                    for t in range(OT)]
            kp_t = [sb.tile([128, S], st_dt, tag=f"kp{t}", name=f"kp{t}")
                    for t in range(OT)]
            # v laid out [k-tile x head x (64 v cols + ones col)]
            VW = HL * (DH + 1)
            vp_t = sb.tile([128, KT * VW], st_dt, tag="vp", name="vp")
            bqk_t = sb.tile([128, 2 * OT], F32, tag="bqk", name="bqk")
            bv_t = sb.tile([1, GH], io_dt, tag="bv", name="bv")
            ones_t = sb.tile([1, 128], io_dt, tag="ones", name="ones")

            def w_ld(w_t, w_d, eng):
                # SBUF [128, k*GH + o]  <-  DRAM [(k p) o]
                eng.dma_start(w_t[:].rearrange("p (k o) -> p k o", o=GH),
                              w_d.rearrange("(k p) o -> p k o", p=128))

            x_t = {}

            def x_ld(which, x_d, eng, sc):
                # one chunk (512 s-cols) of one input, laid [128, k*512+s]
                tag = f"x{which}{sc}" if full_x else f"x{which}"
                t = sb.tile([128, KT * 512], io_dt, tag=tag,
                            name=f"x{which}_{sc}")
                eng.dma_start(
                    t[:].rearrange("p (k s) -> p k s", s=512),
                    x_d.rearrange("(k p) s -> p k s",
                                  p=128)[:, :, sc * 512:(sc + 1) * 512])
                x_t[(which, sc)] = t

            def xap(which, sc, k):
                return x_t[(which, sc)][:, k * 512:(k + 1) * 512]

            nc.sync.dma_start(bv_t[:], bv_d)
            nc.sync.dma_start(ones_t[:], ones_d[0:1, :])
            nc.sync.dma_start(bqk_t[:], bqk_d)
            ones64_t = sb.tile([128, KT * HL], io_dt, tag="ones64",
                               name="ones64")
            nc.sync.dma_start(ones64_t[:], ones_d[:, 0:KT * HL])
            x_ld("q", xq_d, nc.scalar, 0)
            w_ld(wq_t, wq_d, nc.sync)
            x_ld("k", xk_d, nc.sync, 0)
            w_ld(wk_t, wk_d, nc.scalar)
            x_ld("v", xv_d, nc.scalar, 0)
            w_ld(wv_t, wv_d, nc.sync)

            # HAM warmup: ~7us of tiny matmuls on early-arriving const tiles
            # so the real matmuls start at 2.4GHz instead of 1.2
            warm = ps.tile([128, 512], F32, tag="av", bufs=2, name="warm")
            for i in range(20):
                nc.tensor.matmul(warm[:], _mm(ones_t[:], cast),
                                 _mm(bv_t[:], cast), start=True, stop=True)
            if full_x:
                x_ld("q", xq_d, nc.sync, 1)
                x_ld("k", xk_d, nc.scalar, 1)
                x_ld("v", xv_d, nc.scalar, 1)
            v4 = vp_t[:].rearrange("p (k n c) -> p k n c", n=HL, c=DH + 1)
            nc.vector.tensor_copy(
                v4[:, :, :, DH:DH + 1],
                ones64_t[:].rearrange("p (k n one) -> p k n one", n=HL, one=1))

            def proj_qk(sc, ot):
                """one o-tile, one s-chunk of the transposed q/k projections"""
                for which, w_t, xw, o_t in (("q", wq_t, "q", qp_t),
                                            ("k", wk_t, "k", kp_t)):
                    pp = ps.tile([128, 1024], F32, tag="alpha", bufs=3,
                                 name=f"pp{which}_{sc}_{ot}")
                    for k in range(KT):
                        nc.tensor.matmul(
                            pp[:, 0:512],
                            _mm(w_t[:, k * GH + ot * 128:
                                    k * GH + (ot + 1) * 128], cast),
                            _mm(xap(xw, sc, k), cast),
                            start=(k == 0), stop=(k == KT - 1))
                    wi = 0 if which == "q" else 1
                    bias = bqk_t[:, wi * OT + ot:wi * OT + ot + 1]
                    nc.vector.tensor_scalar(
                        o_t[ot][:, sc * 512:(sc + 1) * 512], pp[:, 0:512],
                        bias, 0.0, mybir.AluOpType.add, mybir.AluOpType.max)

            def proj_v(sc, j):
                """one s-tile (128 rows of vp) within chunk sc"""
                st = sc * 4 + j
                pp = ps.tile([128, 1024], F32, tag="alpha", bufs=3,
                             name=f"ppv_{st}")
                nc.tensor.matmul(pp[:, 0:512], _mm(ones_t[:], cast),
                                 _mm(bv_t[:], cast), start=True, stop=False)
                for k in range(KT):
                    nc.tensor.matmul(
                        pp[:, 0:512],
                        _mm(xap("v", sc, k)[:, j * 128:(j + 1) * 128], cast),
                        _mm(wv_t[:, k * GH:(k + 1) * GH], cast),
                        start=False, stop=(k == KT - 1))
                v3 = vp_t[:, st * VW:(st + 1) * VW].rearrange(
                    "p (n c) -> p n c", c=DH + 1)
                p3 = pp[:, 0:512].rearrange("p (n c) -> p n c", c=DH)
                nc.vector.tensor_scalar(
                    v3[:, :, 0:DH], p3, 0.0, None, mybir.AluOpType.max)

            pt_all = {}

            def alphas(n0):
                """alpha + exp for head pair (n0, n0+1); the two heads live on
                disjoint 64-partition halves of o-tile n0//2, so adjacent
                matmuls target disjoint PE row-groups and overlap."""
                t = n0 // 2
                pts0, pts1 = [], []
                for k in range(KT):
                    apts = []
                    for h in range(2):
                        apt = ps.tile([128, 1024], F32, tag="alpha", bufs=3,
                                      name=f"alp_{n0 + h}_{k}")
                        apts.append(apt)
                    order = ([(qc, h) for qc in range(2) for h in range(2)]
                             if ALPHA_ILV else
                             [(qc, h) for h in range(2) for qc in range(2)])
                    for qc, h in order:
                        off = h * 64
                        nc.tensor.matmul(
                            apts[h][:, qc * 512:(qc + 1) * 512],
                            _mm(kp_t[t][off:off + 64,
                                        k * 128:(k + 1) * 128], cast),
                            _mm(qp_t[t][off:off + 64,
                                        qc * 512:(qc + 1) * 512], cast),
                            start=True, stop=True)
                    for h, pts in ((0, pts0), (1, pts1)):
                        pt = sb.tile([128, 1024], st_dt, tag="pt",
                                     bufs=cfg["pt_bufs"], name=f"pt_{n0 + h}_{k}")
                        nc.scalar.activation(pt[:], apts[h][:],
                                             mybir.ActivationFunctionType.Exp,
                                             scale=SCALE)
                        pts.append(pt)
                pt_all[n0] = pts0
                pt_all[n0 + 1] = pts1

            def head_seq(n):
                """unpaired alpha+exp then AV for one head (low pt_bufs modes)"""
                t, off = n // 2, (n % 2) * 64
                pts = []
                for k in range(KT):
                    apt = ps.tile([128, 1024], F32, tag="alpha", bufs=3,
                                  name=f"alp_{n}_{k}")
                    for qc in range(2):
                        nc.tensor.matmul(
                            apt[:, qc * 512:(qc + 1) * 512],
                            _mm(kp_t[t][off:off + 64,
                                        k * 128:(k + 1) * 128], cast),
                            _mm(qp_t[t][off:off + 64,
                                        qc * 512:(qc + 1) * 512], cast),
                            start=True, stop=True)
                    pt = sb.tile([128, 1024], st_dt, tag="pt",
                                 bufs=cfg["pt_bufs"], name=f"pt_{n}_{k}")
                    nc.scalar.activation(pt[:], apt[:],
                                         mybir.ActivationFunctionType.Exp,
                                         scale=SCALE)
                    pts.append(pt)
                pt_all[n] = pts

            def avs(n):
                pts = pt_all.pop(n)
                hid_t = sb.tile([DH + 1, S], F32, tag="hid",
                                bufs=cfg["hid_bufs"], name=f"hid_{n}")
                for qc in range(2):
                    av = ps.tile([DH + 1, 512], F32, tag="av", bufs=2,
                                 name=f"av_{n}_{qc}")
                    for k in range(KT):
                        nc.tensor.matmul(
                            av[:],
                            _mm(vp_t[:, k * VW + n * (DH + 1):
                                     k * VW + (n + 1) * (DH + 1)], cast),
                            _mm(pts[k][:, qc * 512:(qc + 1) * 512], cast),
                            start=(k == 0), stop=(k == KT - 1))
                    nc.vector.tensor_copy(
                        hid_t[:, qc * 512:(qc + 1) * 512], av[:])
                    nc.sync.dma_start(
                        hid_d[n * (DH + 1):(n + 1) * (DH + 1),
                              qc * 512:(qc + 1) * 512],
                        hid_t[:, qc * 512:(qc + 1) * 512])

            # ---- emission schedule ----
            for ot in range(OT):
                proj_qk(0, ot)
            for j in range(4):
                proj_v(0, j)
            if not full_x:
                x_ld("q", xq_d, nc.sync, 1)
                x_ld("k", xk_d, nc.scalar, 1)
                x_ld("v", xv_d, nc.scalar, 1)
            else:
                pass  # chunk-1 loads were emitted upfront
            if cfg["shift_alphas"]:
                proj_qk(1, 0)
                alphas(0)
                for j in range(4):
                    proj_v(1, j)
                proj_qk(1, 1)
                alphas(2)
                avs(0)
                avs(1)
                proj_qk(1, 2)
                alphas(4)
                avs(2)
                avs(3)
                proj_qk(1, 3)
                alphas(6)
                avs(4)
                avs(5)
                avs(6)
                avs(7)
            else:
                proj_qk(1, 0)
                head_seq(0)
                for j in range(4):
                    proj_v(1, j)
                head_seq(1)
                avs(0)
                avs(1)
                for ot in range(1, OT):
                    proj_qk(1, ot)
                    head_seq(2 * ot)
                    avs(2 * ot)
                    head_seq(2 * ot + 1)
                    avs(2 * ot + 1)

    nc.compile()
    return nc


_NC_CACHE = {}


def _get_nc(mode):
    if mode not in _NC_CACHE:
        _NC_CACHE[mode] = build(mode)
    return _NC_CACHE[mode]


def _prep_inputs(inputs, mode):
    cfg = _cfg(mode)
    np_dt = cfg["np_dt"]
    q = np.asarray(inputs["query"], np.float32)
    k = np.asarray(inputs["key"], np.float32)
    v = np.asarray(inputs["value"], np.float32)
    Wq = np.asarray(inputs["Wq"], np.float32)
    Wk = np.asarray(inputs["Wk"], np.float32)
    Wv = np.asarray(inputs["Wv"], np.float32)
    bq = np.asarray(inputs["bq"], np.float32)
    bk = np.asarray(inputs["bk"], np.float32)
    bv = np.asarray(inputs["bv"], np.float32)

    xq = [np.ascontiguousarray(q[b].T).astype(np_dt) for b in range(B)]
    xk = [np.ascontiguousarray(k[b].T).astype(np_dt) for b in range(B)]
    xv = [np.ascontiguousarray(v[b].T).astype(np_dt) for b in range(B)]
    in_maps = []
    for c in range(NCORES):
        b, g = c // GROUPS, c % GROUPS
        sl = slice(g * GH, (g + 1) * GH)
        bqk = np.stack([bq[sl].reshape(OT, 128).T,
                        bk[sl].reshape(OT, 128).T], 1).reshape(128, 2 * OT)
        in_maps.append({
            "xq": xq[b], "xk": xk[b], "xv": xv[b],
            "wq": np.ascontiguousarray(Wq[sl, :].T).astype(np_dt),
            "wk": np.ascontiguousarray(Wk[sl, :].T).astype(np_dt),
            "wv": np.ascontiguousarray(Wv[sl, :].T).astype(np_dt),
            "bqk": np.ascontiguousarray(bqk, dtype=np.float32),
            "bv": np.ascontiguousarray(bv[None, sl]).astype(np_dt),
            "onesd": np.ones((128, 128), np_dt),
        })
    return in_maps


def run(inputs, mode=MODE, trace=False):
    nc = _get_nc(mode)
    in_maps = _prep_inputs(inputs, mode)
    res = bass_utils.run_bass_kernel_spmd(
        nc, in_maps, core_ids=list(range(NCORES)), trace=trace)

    masks = np.asarray(inputs["masks"], np.float32)
    query = np.asarray(inputs["query"], np.float32)
    out = np.empty((B, S, H), np.float32)
    for c in range(NCORES):
        b, g = c // GROUPS, c % GROUPS
        hid = res.results[c]["hid"].reshape(HL, DH + 1, S)
        hT = hid[:, :DH, :]                      # (HL, DH, S)
        se = hid[:, DH, :]                       # (HL, S)
        blk = (hT / se[:, None, :]).transpose(2, 0, 1).reshape(S, GH)
        out[b, :, g * GH:(g + 1) * GH] = blk
    out = out * masks[:, :, None] + query
    return out, res


def kernel(**inputs) -> np.ndarray:
    out, _ = run(inputs)
    return out


# revision 16
# speedup vs baseline: 1.1415x; 1.0047x over previous
"""Multi-head attention (ReLU-gated projections) on 8 Trainium2 NeuronCores.

Problem (hardcoded): B=4, S=1024, H=1024, NH=16, DH=64.
  qp = relu(q @ Wq.T + bq); kp, vp likewise
  alpha = softmax(qh @ kh.T / sqrt(DH)) * mask[q]
  out = (alpha @ vh).reshape(B,S,H) + query

Sharding: 8 cores = 4 batches x 2 head-groups (8 heads / 512 hidden cols each).

Per-core device kernel (all in transposed "hidden-on-partitions" layout):
  stage 1: qpT[o,s], kpT[o,s] (transposed) and vp[s,o] (normal) projections
           with fused bias+relu. Contraction over h via PE; inputs fed
           host-pre-transposed (xT = x.T per batch).
  stage 2: per head: alphaT[k,q] = khT.T @ qhT (K=64); P=exp(alpha/8) on ACT
           (no max subtraction needed: alpha/8 <= ~5); AV via PE with a ones
           column appended to v so row 64 of the output accumulates
           sumexp[q] for free.  Output: unnormalized hidT (64,S) + sumexp (S)
           per head; host divides, applies mask, adds residual.
"""
import sys

sys.path.insert(0, "/opt/trn_rl_repo")

import os
import numpy as np
import ml_dtypes

import concourse.bass as bass
import concourse.tile as tile
from concourse import bacc, mybir
from concourse import bass_utils

if os.environ.get("BASS_LDW_OPT", "0") == "1":
    _orig_run_command = bass_utils.run_command

    def _patched_run_command(cmd, **kw):
        cmd = ["--enable-ldw-opt=true" if c == "--enable-ldw-opt=false" else c
               for c in cmd]
        return _orig_run_command(cmd, **kw)

    bass_utils.run_command = _patched_run_command

B, S, H = 4, 1024, 1024
NH, DH = 16, 64
NCORES = 8
GROUPS = 2          # head-groups (tensor-parallel dim)
HL = NH // GROUPS   # heads per core = 8
GH = H // GROUPS    # hidden cols per core = 512
KT = H // 128       # contraction k-tiles = 8
OT = GH // 128      # output o-tiles per core = 4
SCALE = 1.0 / float(np.sqrt(DH))

# matmul precision mode: "f32" (exact, 4 cyc/row), "f32r" (TF32-ish, 1 cyc/row),
# "bf16" (1 cyc/row, smallest footprint)
MODE = os.environ.get("BASS_MM_DT", "f32r")
ALPHA_ILV = os.environ.get("BASS_ALPHA_ILV", "1") == "1"

F32 = mybir.dt.float32
F32R = mybir.dt.float32r
BF16 = mybir.dt.bfloat16


def _cfg(mode):
    if mode == "bf16":
        return dict(np_dt=ml_dtypes.bfloat16, io_dt=BF16, st_dt=BF16,
                    cast=False, pt_bufs=36, hid_bufs=3, x_bufs=16,
                    shift_alphas=True, kz=True)
    if mode == "f32r":
        # float32r end-to-end: walrus requires f32r matmul inputs to be
        # *produced* as f32r (DMA loads + DVE/ACT evacuations), not bitcast.
        return dict(np_dt=np.float32, io_dt=F32R, st_dt=F32R,
                    cast=False, pt_bufs=9, hid_bufs=2, x_bufs=8,
                    shift_alphas=False, kz=False)
    return dict(np_dt=np.float32, io_dt=F32, st_dt=F32,
                cast=False, pt_bufs=9, hid_bufs=2, x_bufs=8,
                shift_alphas=False, kz=False)


def _mm(ap, cast):
    return ap.bitcast(F32R) if cast else ap


def build(mode):
    cfg = _cfg(mode)
    io_dt, st_dt, cast = cfg["io_dt"], cfg["st_dt"], cfg["cast"]
    nc = bacc.Bacc("TRN2", target_bir_lowering=False, debug=False,
                   num_devices=NCORES)

    xq_d = nc.dram_tensor("xq", [H, S], io_dt, kind="ExternalInput").ap()
    xk_d = nc.dram_tensor("xk", [H, S], io_dt, kind="ExternalInput").ap()
    xv_d = nc.dram_tensor("xv", [H, S], io_dt, kind="ExternalInput").ap()
    wq_d = nc.dram_tensor("wq", [H, GH], io_dt, kind="ExternalInput").ap()
    wk_d = nc.dram_tensor("wk", [H, GH], io_dt, kind="ExternalInput").ap()
    wv_d = nc.dram_tensor("wv", [H, GH], io_dt, kind="ExternalInput").ap()
    bqk_d = nc.dram_tensor("bqk", [128, 2 * OT], F32, kind="ExternalInput").ap()
    bv_d = nc.dram_tensor("bv", [1, GH], io_dt, kind="ExternalInput").ap()
    ones_d = nc.dram_tensor("onesd", [128, 128], io_dt,
                            kind="ExternalInput").ap()
    zeros_d = nc.dram_tensor("zerosd", [64, S], io_dt,
                             kind="ExternalInput").ap()
    hid_d = nc.dram_tensor("hid", [HL * (DH + 1), S], F32,
                           kind="ExternalOutput").ap()

    with tile.TileContext(nc) as tc:
        with tc.tile_pool(name="sb", bufs=1) as sb, \
             tc.tile_pool(name="ps", bufs=1, space="PSUM") as ps:

            full_x = mode == "bf16"   # x resident for full S vs per-chunk

            # ---- persistent tiles; one big DMA per tensor (>=1MB, descriptor
            #      runs of 1-2KB/partition), spread across the three DGE rings
            #      (sync / scalar / gpsimd) so loads overlap ----
            wq_t = sb.tile([128, KT * GH], io_dt, tag="wq", name="wq")
            wk_t = sb.tile([128, KT * GH], io_dt, tag="wk", name="wk")
            wv_t = sb.tile([128, KT * GH], io_dt, tag="wv", name="wv")
            qp_t = [sb.tile([128, S], st_dt, tag=f"qp{t}", name=f"qp{t}")
                    for t in range(OT)]
            KZ = cfg["kz"]
            if KZ:
                # zero-padded K copies: kz[t][h] holds head h's kh rows in its
                # own 64-partition half, zeros in the other -> full-K=128
                # alpha matmuls whose weight loads pipeline like any other MM
                kz_t = [[sb.tile([128, S], st_dt, tag=f"kz{t}{h}",
                                 name=f"kz{t}{h}") for h in range(2)]
                        for t in range(OT)]
                for t in range(OT):
                    nc.scalar.dma_start(kz_t[t][0][64:128, :], zeros_d)
                    nc.scalar.dma_start(kz_t[t][1][0:64, :], zeros_d)
            else:
                kp_t = [sb.tile([128, S], st_dt, tag=f"kp{t}",
                                name=f"kp{t}") for t in range(OT)]
            # v laid out [k-tile x head x (64 v cols + ones col)]
            VW = HL * (DH + 1)
            vp_t = sb.tile([128, KT * VW], st_dt, tag="vp", name="vp")
            bqk_t = sb.tile([128, 2 * OT], F32, tag="bqk", name="bqk")
            bv_t = sb.tile([1, GH], io_dt, tag="bv", name="bv")
            ones_t = sb.tile([1, 128], io_dt, tag="ones", name="ones")

            def w_ld(w_t, w_d, eng):
                # SBUF [128, k*GH + o]  <-  DRAM [(k p) o]
                eng.dma_start(w_t[:].rearrange("p (k o) -> p k o", o=GH),
                              w_d.rearrange("(k p) o -> p k o", p=128))

            x_t = {}

            def x_ld(which, x_d, eng, sc):
                # one chunk (512 s-cols) of one input, laid [128, k*512+s]
                tag = f"x{which}{sc}" if full_x else f"x{which}"
                t = sb.tile([128, KT * 512], io_dt, tag=tag,
                            name=f"x{which}_{sc}")
                eng.dma_start(
                    t[:].rearrange("p (k s) -> p k s", s=512),
                    x_d.rearrange("(k p) s -> p k s",
                                  p=128)[:, :, sc * 512:(sc + 1) * 512])
                x_t[(which, sc)] = t

            def xap(which, sc, k):
                return x_t[(which, sc)][:, k * 512:(k + 1) * 512]

            nc.sync.dma_start(bv_t[:], bv_d)
            nc.sync.dma_start(ones_t[:], ones_d[0:1, :])
            nc.sync.dma_start(bqk_t[:], bqk_d)
            ones64_t = sb.tile([128, KT * HL], io_dt, tag="ones64",
                               name="ones64")
            nc.sync.dma_start(ones64_t[:], ones_d[:, 0:KT * HL])
            x_ld("q", xq_d, nc.scalar, 0)
            w_ld(wq_t, wq_d, nc.sync)
            x_ld("k", xk_d, nc.sync, 0)
            w_ld(wk_t, wk_d, nc.scalar)
            x_ld("v", xv_d, nc.scalar, 0)
            w_ld(wv_t, wv_d, nc.sync)

            # HAM warmup: ~7us of tiny matmuls on early-arriving const tiles
            # so the real matmuls start at 2.4GHz instead of 1.2
            warm = ps.tile([128, 512], F32, tag="av", bufs=2, name="warm")
            for i in range(20):
                nc.tensor.matmul(warm[:], _mm(ones_t[:], cast),
                                 _mm(bv_t[:], cast), start=True, stop=True)
            if full_x:
                x_ld("q", xq_d, nc.sync, 1)
                x_ld("k", xk_d, nc.scalar, 1)
                x_ld("v", xv_d, nc.scalar, 1)
            v4 = vp_t[:].rearrange("p (k n c) -> p k n c", n=HL, c=DH + 1)
            nc.vector.tensor_copy(
                v4[:, :, :, DH:DH + 1],
                ones64_t[:].rearrange("p (k n one) -> p k n one", n=HL, one=1))

            def proj_qk(sc, ot):
                """one o-tile, one s-chunk of the transposed q/k projections"""
                for which, w_t, xw in (("q", wq_t, "q"), ("k", wk_t, "k")):
                    pp = ps.tile([128, 1024], F32, tag="alpha", bufs=3,
                                 name=f"pp{which}_{sc}_{ot}")
                    for k in range(KT):
                        nc.tensor.matmul(
                            pp[:, 0:512],
                            _mm(w_t[:, k * GH + ot * 128:
                                    k * GH + (ot + 1) * 128], cast),
                            _mm(xap(xw, sc, k), cast),
                            start=(k == 0), stop=(k == KT - 1))
                    wi = 0 if which == "q" else 1
                    bias = bqk_t[:, wi * OT + ot:wi * OT + ot + 1]
                    ssl = slice(sc * 512, (sc + 1) * 512)
                    if which == "q":
                        nc.vector.tensor_scalar(
                            qp_t[ot][:, ssl], pp[:, 0:512], bias, 0.0,
                            mybir.AluOpType.add, mybir.AluOpType.max)
                    elif KZ:
                        for h in range(2):
                            pr = slice(h * 64, h * 64 + 64)
                            nc.vector.tensor_scalar(
                                kz_t[ot][h][pr, ssl], pp[pr, 0:512],
                                bias[pr, :], 0.0,
                                mybir.AluOpType.add, mybir.AluOpType.max)
                    else:
                        nc.vector.tensor_scalar(
                            kp_t[ot][:, ssl], pp[:, 0:512], bias, 0.0,
                            mybir.AluOpType.add, mybir.AluOpType.max)

            def proj_v(sc, j):
                """one s-tile (128 rows of vp) within chunk sc"""
                st = sc * 4 + j
                pp = ps.tile([128, 1024], F32, tag="alpha", bufs=3,
                             name=f"ppv_{st}")
                nc.tensor.matmul(pp[:, 0:512], _mm(ones_t[:], cast),
                                 _mm(bv_t[:], cast), start=True, stop=False)
                for k in range(KT):
                    nc.tensor.matmul(
                        pp[:, 0:512],
                        _mm(xap("v", sc, k)[:, j * 128:(j + 1) * 128], cast),
                        _mm(wv_t[:, k * GH:(k + 1) * GH], cast),
                        start=False, stop=(k == KT - 1))
                v3 = vp_t[:, st * VW:(st + 1) * VW].rearrange(
                    "p (n c) -> p n c", c=DH + 1)
                p3 = pp[:, 0:512].rearrange("p (n c) -> p n c", c=DH)
                nc.vector.tensor_scalar(
                    v3[:, :, 0:DH], p3, 0.0, None, mybir.AluOpType.max)

            pt_all = {}

            def alphas(n0):
                """alpha + exp for head pair (n0, n0+1); the two heads live on
                disjoint 64-partition halves of o-tile n0//2, so adjacent
                matmuls target disjoint PE row-groups and overlap."""
                t = n0 // 2
                pts0, pts1 = [], []
                for k in range(KT):
                    apts = []
                    for h in range(2):
                        apt = ps.tile([128, 1024], F32, tag="alpha", bufs=3,
                                      name=f"alp_{n0 + h}_{k}")
                        apts.append(apt)
                    for qc in range(2):
                        for h in range(2):
                            nc.tensor.matmul(
                                apts[h][:, qc * 512:(qc + 1) * 512],
                                _mm(kz_t[t][h][:, k * 128:(k + 1) * 128],
                                    cast),
                                _mm(qp_t[t][:, qc * 512:(qc + 1) * 512],
                                    cast),
                                start=True, stop=True)
                    for h, pts in ((0, pts0), (1, pts1)):
                        pt = sb.tile([128, 1024], st_dt, tag="pt",
                                     bufs=cfg["pt_bufs"], name=f"pt_{n0 + h}_{k}")
                        nc.scalar.activation(pt[:], apts[h][:],
                                             mybir.ActivationFunctionType.Exp,
                                             scale=SCALE)
                        pts.append(pt)
                pt_all[n0] = pts0
                pt_all[n0 + 1] = pts1

            def head_seq(n):
                """unpaired alpha+exp then AV for one head (low pt_bufs modes)"""
                t, off = n // 2, (n % 2) * 64
                pts = []
                for k in range(KT):
                    apt = ps.tile([128, 1024], F32, tag="alpha", bufs=3,
                                  name=f"alp_{n}_{k}")
                    for qc in range(2):
                        nc.tensor.matmul(
                            apt[:, qc * 512:(qc + 1) * 512],
                            _mm(kp_t[t][off:off + 64,
                                        k * 128:(k + 1) * 128], cast),
                            _mm(qp_t[t][off:off + 64,
                                        qc * 512:(qc + 1) * 512], cast),
                            start=True, stop=True)
                    pt = sb.tile([128, 1024], st_dt, tag="pt",
                                 bufs=cfg["pt_bufs"], name=f"pt_{n}_{k}")
                    nc.scalar.activation(pt[:], apt[:],
                                         mybir.ActivationFunctionType.Exp,
                                         scale=SCALE)
                    pts.append(pt)
                pt_all[n] = pts

            def avs(n):
                pts = pt_all.pop(n)
                hid_t = sb.tile([DH + 1, S], F32, tag="hid",
                                bufs=cfg["hid_bufs"], name=f"hid_{n}")
                for qc in range(2):
                    av = ps.tile([DH + 1, 512], F32, tag="av", bufs=2,
                                 name=f"av_{n}_{qc}")
                    for k in range(KT):
                        nc.tensor.matmul(
                            av[:],
                            _mm(vp_t[:, k * VW + n * (DH + 1):
                                     k * VW + (n + 1) * (DH + 1)], cast),
                            _mm(pts[k][:, qc * 512:(qc + 1) * 512], cast),
                            start=(k == 0), stop=(k == KT - 1))
                    nc.vector.tensor_copy(
                        hid_t[:, qc * 512:(qc + 1) * 512], av[:])
                    nc.sync.dma_start(
                        hid_d[n * (DH + 1):(n + 1) * (DH + 1),
                              qc * 512:(qc + 1) * 512],
                        hid_t[:, qc * 512:(qc + 1) * 512])

            # ---- emission schedule ----
            for ot in range(OT):
                proj_qk(0, ot)
            for j in range(4):
                proj_v(0, j)
            if not full_x:
                x_ld("q", xq_d, nc.sync, 1)
                x_ld("k", xk_d, nc.scalar, 1)
                x_ld("v", xv_d, nc.scalar, 1)
            else:
                pass  # chunk-1 loads were emitted upfront
            if cfg["shift_alphas"]:
                proj_qk(1, 0)
                alphas(0)
                for j in range(4):
                    proj_v(1, j)
                proj_qk(1, 1)
                alphas(2)
                avs(0)
                avs(1)
                proj_qk(1, 2)
                alphas(4)
                avs(2)
                avs(3)
                proj_qk(1, 3)
                alphas(6)
                avs(4)
                avs(5)
                avs(6)
                avs(7)
            else:
                proj_qk(1, 0)
                head_seq(0)
                for j in range(4):
                    proj_v(1, j)
                head_seq(1)
                avs(0)
                avs(1)
                for ot in range(1, OT):
                    proj_qk(1, ot)
                    head_seq(2 * ot)
                    avs(2 * ot)
                    head_seq(2 * ot + 1)
                    avs(2 * ot + 1)

    nc.compile()
    return nc


_NC_CACHE = {}


def _get_nc(mode):
    if mode not in _NC_CACHE:
        _NC_CACHE[mode] = build(mode)
    return _NC_CACHE[mode]


def _prep_inputs(inputs, mode):
    cfg = _cfg(mode)
    np_dt = cfg["np_dt"]
    q = np.asarray(inputs["query"], np.float32)
    k = np.asarray(inputs["key"], np.float32)
    v = np.asarray(inputs["value"], np.float32)
    Wq = np.asarray(inputs["Wq"], np.float32)
    Wk = np.asarray(inputs["Wk"], np.float32)
    Wv = np.asarray(inputs["Wv"], np.float32)
    bq = np.asarray(inputs["bq"], np.float32)
    bk = np.asarray(inputs["bk"], np.float32)
    bv = np.asarray(inputs["bv"], np.float32)

    xq = [np.ascontiguousarray(q[b].T).astype(np_dt) for b in range(B)]
    xk = [np.ascontiguousarray(k[b].T).astype(np_dt) for b in range(B)]
    xv = [np.ascontiguousarray(v[b].T).astype(np_dt) for b in range(B)]
    in_maps = []
    for c in range(NCORES):
        b, g = c // GROUPS, c % GROUPS
        sl = slice(g * GH, (g + 1) * GH)
        bqk = np.stack([bq[sl].reshape(OT, 128).T,
                        bk[sl].reshape(OT, 128).T], 1).reshape(128, 2 * OT)
        in_maps.append({
            "xq": xq[b], "xk": xk[b], "xv": xv[b],
            "wq": np.ascontiguousarray(Wq[sl, :].T).astype(np_dt),
            "wk": np.ascontiguousarray(Wk[sl, :].T).astype(np_dt),
            "wv": np.ascontiguousarray(Wv[sl, :].T).astype(np_dt),
            "bqk": np.ascontiguousarray(bqk, dtype=np.float32),
            "bv": np.ascontiguousarray(bv[None, sl]).astype(np_dt),
            "onesd": np.ones((128, 128), np_dt),
            "zerosd": np.zeros((64, S), np_dt),
        })
    return in_maps


def run(inputs, mode=MODE, trace=False):
    nc = _get_nc(mode)
    in_maps = _prep_inputs(inputs, mode)
    res = bass_utils.run_bass_kernel_spmd(
        nc, in_maps, core_ids=list(range(NCORES)), trace=trace)

    masks = np.asarray(inputs["masks"], np.float32)
    query = np.asarray(inputs["query"], np.float32)
    out = np.empty((B, S, H), np.float32)
    for c in range(NCORES):
        b, g = c // GROUPS, c % GROUPS
        hid = res.results[c]["hid"].reshape(HL, DH + 1, S)
        hT = hid[:, :DH, :]                      # (HL, DH, S)
        se = hid[:, DH, :]                       # (HL, S)
        blk = (hT / se[:, None, :]).transpose(2, 0, 1).reshape(S, GH)
        out[b, :, g * GH:(g + 1) * GH] = blk
    out = out * masks[:, :, None] + query
    return out, res


def kernel(**inputs) -> np.ndarray:
    out, _ = run(inputs)
    return out


# revision 17
# speedup vs baseline: 1.1557x; 1.0125x over previous
"""Multi-head attention (ReLU-gated projections) on 8 Trainium2 NeuronCores.

Problem (hardcoded): B=4, S=1024, H=1024, NH=16, DH=64.
  qp = relu(q @ Wq.T + bq); kp, vp likewise
  alpha = softmax(qh @ kh.T / sqrt(DH)) * mask[q]
  out = (alpha @ vh).reshape(B,S,H) + query

Sharding: 8 cores = 4 batches x 2 head-groups (8 heads / 512 hidden cols each).

Per-core device kernel (all in transposed "hidden-on-partitions" layout):
  stage 1: qpT[o,s], kpT[o,s] (transposed) and vp[s,o] (normal) projections
           with fused bias+relu. Contraction over h via PE; inputs fed
           host-pre-transposed (xT = x.T per batch).
  stage 2: per head: alphaT[k,q] = khT.T @ qhT (K=64); P=exp(alpha/8) on ACT
           (no max subtraction needed: alpha/8 <= ~5); AV via PE with a ones
           column appended to v so row 64 of the output accumulates
           sumexp[q] for free.  Output: unnormalized hidT (64,S) + sumexp (S)
           per head; host divides, applies mask, adds residual.
"""
import sys

sys.path.insert(0, "/opt/trn_rl_repo")

import os
import numpy as np
import ml_dtypes

import concourse.bass as bass
import concourse.tile as tile
from concourse import bacc, mybir
from concourse import bass_utils

if os.environ.get("BASS_LDW_OPT", "0") == "1":
    _orig_run_command = bass_utils.run_command

    def _patched_run_command(cmd, **kw):
        cmd = ["--enable-ldw-opt=true" if c == "--enable-ldw-opt=false" else c
               for c in cmd]
        return _orig_run_command(cmd, **kw)

    bass_utils.run_command = _patched_run_command

B, S, H = 4, 1024, 1024
NH, DH = 16, 64
NCORES = 8
GROUPS = 2          # head-groups (tensor-parallel dim)
HL = NH // GROUPS   # heads per core = 8
GH = H // GROUPS    # hidden cols per core = 512
KT = H // 128       # contraction k-tiles = 8
OT = GH // 128      # output o-tiles per core = 4
SCALE = 1.0 / float(np.sqrt(DH))

# matmul precision mode: "f32" (exact, 4 cyc/row), "f32r" (TF32-ish, 1 cyc/row),
# "bf16" (1 cyc/row, smallest footprint)
MODE = os.environ.get("BASS_MM_DT", "f32r")
ALPHA_ILV = os.environ.get("BASS_ALPHA_ILV", "1") == "1"

F32 = mybir.dt.float32
F32R = mybir.dt.float32r
BF16 = mybir.dt.bfloat16


def _cfg(mode):
    if mode == "bf16":
        return dict(np_dt=ml_dtypes.bfloat16, io_dt=BF16, st_dt=BF16,
                    cast=False, pt_bufs=36, hid_bufs=3, x_bufs=16,
                    shift_alphas=True, kz=True)
    if mode == "f32r":
        # float32r end-to-end: walrus requires f32r matmul inputs to be
        # *produced* as f32r (DMA loads + DVE/ACT evacuations), not bitcast.
        return dict(np_dt=np.float32, io_dt=F32R, st_dt=F32R,
                    cast=False, pt_bufs=9, hid_bufs=2, x_bufs=8,
                    shift_alphas=False, kz=False)
    return dict(np_dt=np.float32, io_dt=F32, st_dt=F32,
                cast=False, pt_bufs=9, hid_bufs=2, x_bufs=8,
                shift_alphas=False, kz=False)


def _mm(ap, cast):
    return ap.bitcast(F32R) if cast else ap


def build(mode):
    cfg = _cfg(mode)
    io_dt, st_dt, cast = cfg["io_dt"], cfg["st_dt"], cfg["cast"]
    nc = bacc.Bacc("TRN2", target_bir_lowering=False, debug=False,
                   num_devices=NCORES)

    xq_d = nc.dram_tensor("xq", [H, S], io_dt, kind="ExternalInput").ap()
    xk_d = nc.dram_tensor("xk", [H, S], io_dt, kind="ExternalInput").ap()
    xv_d = nc.dram_tensor("xv", [H, S], io_dt, kind="ExternalInput").ap()
    wq_d = nc.dram_tensor("wq", [H, GH], io_dt, kind="ExternalInput").ap()
    wk_d = nc.dram_tensor("wk", [H, GH], io_dt, kind="ExternalInput").ap()
    wv_d = nc.dram_tensor("wv", [H, GH], io_dt, kind="ExternalInput").ap()
    bqk_d = nc.dram_tensor("bqk", [128, 2 * OT], F32, kind="ExternalInput").ap()
    bv_d = nc.dram_tensor("bv", [1, GH], io_dt, kind="ExternalInput").ap()
    ones_d = nc.dram_tensor("onesd", [128, 128], io_dt,
                            kind="ExternalInput").ap()
    zeros_d = nc.dram_tensor("zerosd", [64, S], io_dt,
                             kind="ExternalInput").ap()
    hid_d = nc.dram_tensor("hid", [HL * (DH + 1), S], F32,
                           kind="ExternalOutput").ap()

    with tile.TileContext(nc) as tc:
        with tc.tile_pool(name="sb", bufs=1) as sb, \
             tc.tile_pool(name="ps", bufs=1, space="PSUM") as ps:

            full_x = mode == "bf16"   # x resident for full S vs per-chunk

            # ---- persistent tiles; one big DMA per tensor (>=1MB, descriptor
            #      runs of 1-2KB/partition), spread across the three DGE rings
            #      (sync / scalar / gpsimd) so loads overlap ----
            wq_t = sb.tile([128, KT * GH], io_dt, tag="wq", name="wq")
            wk_t = sb.tile([128, KT * GH], io_dt, tag="wk", name="wk")
            wv_t = sb.tile([128, KT * GH], io_dt, tag="wv", name="wv")
            qp_t = [sb.tile([128, S], st_dt, tag=f"qp{t}", name=f"qp{t}")
                    for t in range(OT)]
            KZ = cfg["kz"]
            if KZ:
                # zero-padded K copies: kz[t][h] holds head h's kh rows in its
                # own 64-partition half, zeros in the other -> full-K=128
                # alpha matmuls whose weight loads pipeline like any other MM
                kz_t = [[sb.tile([128, S], st_dt, tag=f"kz{t}{h}",
                                 name=f"kz{t}{h}") for h in range(2)]
                        for t in range(OT)]
                kz_zero_dmas = [(t, h) for t in range(OT) for h in range(2)]
            else:
                kp_t = [sb.tile([128, S], st_dt, tag=f"kp{t}",
                                name=f"kp{t}") for t in range(OT)]
            # v laid out [k-tile x head x (64 v cols + ones col)]
            VW = HL * (DH + 1)
            vp_t = sb.tile([128, KT * VW], st_dt, tag="vp", name="vp")
            bqk_t = sb.tile([128, 2 * OT], F32, tag="bqk", name="bqk")
            bv_t = sb.tile([1, GH], io_dt, tag="bv", name="bv")
            ones_t = sb.tile([1, 128], io_dt, tag="ones", name="ones")

            def w_ld(w_t, w_d, eng):
                # SBUF [128, k*GH + o]  <-  DRAM [(k p) o]
                eng.dma_start(w_t[:].rearrange("p (k o) -> p k o", o=GH),
                              w_d.rearrange("(k p) o -> p k o", p=128))

            x_t = {}

            def x_ld(which, x_d, eng, sc):
                # one chunk (512 s-cols) of one input, laid [128, k*512+s]
                tag = f"x{which}{sc}" if full_x else f"x{which}"
                t = sb.tile([128, KT * 512], io_dt, tag=tag,
                            name=f"x{which}_{sc}")
                eng.dma_start(
                    t[:].rearrange("p (k s) -> p k s", s=512),
                    x_d.rearrange("(k p) s -> p k s",
                                  p=128)[:, :, sc * 512:(sc + 1) * 512])
                x_t[(which, sc)] = t

            def xap(which, sc, k):
                return x_t[(which, sc)][:, k * 512:(k + 1) * 512]

            nc.sync.dma_start(bv_t[:], bv_d)
            nc.sync.dma_start(ones_t[:], ones_d[0:1, :])
            nc.sync.dma_start(bqk_t[:], bqk_d)
            ones64_t = sb.tile([128, KT * HL], io_dt, tag="ones64",
                               name="ones64")
            nc.sync.dma_start(ones64_t[:], ones_d[:, 0:KT * HL])
            x_ld("q", xq_d, nc.scalar, 0)
            w_ld(wq_t, wq_d, nc.sync)
            x_ld("k", xk_d, nc.sync, 0)
            w_ld(wk_t, wk_d, nc.scalar)
            x_ld("v", xv_d, nc.scalar, 0)
            w_ld(wv_t, wv_d, nc.sync)

            # HAM warmup: ~7us of tiny matmuls on early-arriving const tiles
            # so the real matmuls start at 2.4GHz instead of 1.2
            warm = ps.tile([128, 512], F32, tag="av", bufs=2, name="warm")
            for i in range(20):
                nc.tensor.matmul(warm[:], _mm(ones_t[:], cast),
                                 _mm(bv_t[:], cast), start=True, stop=True)
            if full_x:
                x_ld("q", xq_d, nc.sync, 1)
                x_ld("k", xk_d, nc.gpsimd, 1)
                x_ld("v", xv_d, nc.gpsimd, 1)
                for t, h in kz_zero_dmas:
                    nc.sync.dma_start(
                        kz_t[t][h][64 * (1 - h):64 * (1 - h) + 64, :], zeros_d)
            v4 = vp_t[:].rearrange("p (k n c) -> p k n c", n=HL, c=DH + 1)
            nc.vector.tensor_copy(
                v4[:, :, :, DH:DH + 1],
                ones64_t[:].rearrange("p (k n one) -> p k n one", n=HL, one=1))

            def proj_qk(sc, ot):
                """one o-tile, one s-chunk of the transposed q/k projections"""
                for which, w_t, xw in (("q", wq_t, "q"), ("k", wk_t, "k")):
                    pp = ps.tile([128, 1024], F32, tag="alpha", bufs=3,
                                 name=f"pp{which}_{sc}_{ot}")
                    for k in range(KT):
                        nc.tensor.matmul(
                            pp[:, 0:512],
                            _mm(w_t[:, k * GH + ot * 128:
                                    k * GH + (ot + 1) * 128], cast),
                            _mm(xap(xw, sc, k), cast),
                            start=(k == 0), stop=(k == KT - 1))
                    wi = 0 if which == "q" else 1
                    bias = bqk_t[:, wi * OT + ot:wi * OT + ot + 1]
                    ssl = slice(sc * 512, (sc + 1) * 512)
                    if which == "q":
                        nc.vector.tensor_scalar(
                            qp_t[ot][:, ssl], pp[:, 0:512], bias, 0.0,
                            mybir.AluOpType.add, mybir.AluOpType.max)
                    elif KZ:
                        for h in range(2):
                            pr = slice(h * 64, h * 64 + 64)
                            nc.vector.tensor_scalar(
                                kz_t[ot][h][pr, ssl], pp[pr, 0:512],
                                bias[pr, :], 0.0,
                                mybir.AluOpType.add, mybir.AluOpType.max)
                    else:
                        nc.vector.tensor_scalar(
                            kp_t[ot][:, ssl], pp[:, 0:512], bias, 0.0,
                            mybir.AluOpType.add, mybir.AluOpType.max)

            def proj_v(sc, j):
                """one s-tile (128 rows of vp) within chunk sc"""
                st = sc * 4 + j
                pp = ps.tile([128, 1024], F32, tag="alpha", bufs=3,
                             name=f"ppv_{st}")
                nc.tensor.matmul(pp[:, 0:512], _mm(ones_t[:], cast),
                                 _mm(bv_t[:], cast), start=True, stop=False)
                for k in range(KT):
                    nc.tensor.matmul(
                        pp[:, 0:512],
                        _mm(xap("v", sc, k)[:, j * 128:(j + 1) * 128], cast),
                        _mm(wv_t[:, k * GH:(k + 1) * GH], cast),
                        start=False, stop=(k == KT - 1))
                v3 = vp_t[:, st * VW:(st + 1) * VW].rearrange(
                    "p (n c) -> p n c", c=DH + 1)
                p3 = pp[:, 0:512].rearrange("p (n c) -> p n c", c=DH)
                nc.vector.tensor_scalar(
                    v3[:, :, 0:DH], p3, 0.0, None, mybir.AluOpType.max)

            pt_all = {}

            def alphas(n0):
                """alpha + exp for head pair (n0, n0+1); the two heads live on
                disjoint 64-partition halves of o-tile n0//2, so adjacent
                matmuls target disjoint PE row-groups and overlap."""
                t = n0 // 2
                pts0, pts1 = [], []
                for k in range(KT):
                    apts = []
                    for h in range(2):
                        apt = ps.tile([128, 1024], F32, tag="alpha", bufs=3,
                                      name=f"alp_{n0 + h}_{k}")
                        apts.append(apt)
                    for qc in range(2):
                        for h in range(2):
                            nc.tensor.matmul(
                                apts[h][:, qc * 512:(qc + 1) * 512],
                                _mm(kz_t[t][h][:, k * 128:(k + 1) * 128],
                                    cast),
                                _mm(qp_t[t][:, qc * 512:(qc + 1) * 512],
                                    cast),
                                start=True, stop=True)
                    for h, pts in ((0, pts0), (1, pts1)):
                        pt = sb.tile([128, 1024], st_dt, tag="pt",
                                     bufs=cfg["pt_bufs"], name=f"pt_{n0 + h}_{k}")
                        nc.scalar.activation(pt[:], apts[h][:],
                                             mybir.ActivationFunctionType.Exp,
                                             scale=SCALE)
                        pts.append(pt)
                pt_all[n0] = pts0
                pt_all[n0 + 1] = pts1

            def head_seq(n):
                """unpaired alpha+exp then AV for one head (low pt_bufs modes)"""
                t, off = n // 2, (n % 2) * 64
                pts = []
                for k in range(KT):
                    apt = ps.tile([128, 1024], F32, tag="alpha", bufs=3,
                                  name=f"alp_{n}_{k}")
                    for qc in range(2):
                        nc.tensor.matmul(
                            apt[:, qc * 512:(qc + 1) * 512],
                            _mm(kp_t[t][off:off + 64,
                                        k * 128:(k + 1) * 128], cast),
                            _mm(qp_t[t][off:off + 64,
                                        qc * 512:(qc + 1) * 512], cast),
                            start=True, stop=True)
                    pt = sb.tile([128, 1024], st_dt, tag="pt",
                                 bufs=cfg["pt_bufs"], name=f"pt_{n}_{k}")
                    nc.scalar.activation(pt[:], apt[:],
                                         mybir.ActivationFunctionType.Exp,
                                         scale=SCALE)
                    pts.append(pt)
                pt_all[n] = pts

            def avs(n):
                pts = pt_all.pop(n)
                hid_t = sb.tile([DH + 1, S], F32, tag="hid",
                                bufs=cfg["hid_bufs"], name=f"hid_{n}")
                for qc in range(2):
                    av = ps.tile([DH + 1, 512], F32, tag="av", bufs=2,
                                 name=f"av_{n}_{qc}")
                    for k in range(KT):
                        nc.tensor.matmul(
                            av[:],
                            _mm(vp_t[:, k * VW + n * (DH + 1):
                                     k * VW + (n + 1) * (DH + 1)], cast),
                            _mm(pts[k][:, qc * 512:(qc + 1) * 512], cast),
                            start=(k == 0), stop=(k == KT - 1))
                    nc.vector.tensor_copy(
                        hid_t[:, qc * 512:(qc + 1) * 512], av[:])
                    nc.sync.dma_start(
                        hid_d[n * (DH + 1):(n + 1) * (DH + 1),
                              qc * 512:(qc + 1) * 512],
                        hid_t[:, qc * 512:(qc + 1) * 512])

            # ---- emission schedule ----
            for ot in range(OT):
                proj_qk(0, ot)
            for j in range(4):
                proj_v(0, j)
            if not full_x:
                x_ld("q", xq_d, nc.sync, 1)
                x_ld("k", xk_d, nc.gpsimd, 1)
                x_ld("v", xv_d, nc.gpsimd, 1)
            if cfg["shift_alphas"]:
                proj_qk(1, 0)
                alphas(0)
                for j in range(4):
                    proj_v(1, j)
                proj_qk(1, 1)
                alphas(2)
                avs(0)
                avs(1)
                proj_qk(1, 2)
                alphas(4)
                avs(2)
                avs(3)
                proj_qk(1, 3)
                alphas(6)
                avs(4)
                avs(5)
                avs(6)
                avs(7)
            else:
                proj_qk(1, 0)
                head_seq(0)
                for j in range(4):
                    proj_v(1, j)
                head_seq(1)
                avs(0)
                avs(1)
                for ot in range(1, OT):
                    proj_qk(1, ot)
                    head_seq(2 * ot)
                    avs(2 * ot)
                    head_seq(2 * ot + 1)
                    avs(2 * ot + 1)

    nc.compile()
    return nc


_NC_CACHE = {}


def _get_nc(mode):
    if mode not in _NC_CACHE:
        _NC_CACHE[mode] = build(mode)
    return _NC_CACHE[mode]


def _prep_inputs(inputs, mode):
    cfg = _cfg(mode)
    np_dt = cfg["np_dt"]
    q = np.asarray(inputs["query"], np.float32)
    k = np.asarray(inputs["key"], np.float32)
    v = np.asarray(inputs["value"], np.float32)
    Wq = np.asarray(inputs["Wq"], np.float32)
    Wk = np.asarray(inputs["Wk"], np.float32)
    Wv = np.asarray(inputs["Wv"], np.float32)
    bq = np.asarray(inputs["bq"], np.float32)
    bk = np.asarray(inputs["bk"], np.float32)
    bv = np.asarray(inputs["bv"], np.float32)

    xq = [np.ascontiguousarray(q[b].T).astype(np_dt) for b in range(B)]
    xk = [np.ascontiguousarray(k[b].T).astype(np_dt) for b in range(B)]
    xv = [np.ascontiguousarray(v[b].T).astype(np_dt) for b in range(B)]
    in_maps = []
    for c in range(NCORES):
        b, g = c // GROUPS, c % GROUPS
        sl = slice(g * GH, (g + 1) * GH)
        bqk = np.stack([bq[sl].reshape(OT, 128).T,
                        bk[sl].reshape(OT, 128).T], 1).reshape(128, 2 * OT)
        in_maps.append({
            "xq": xq[b], "xk": xk[b], "xv": xv[b],
            "wq": np.ascontiguousarray(Wq[sl, :].T).astype(np_dt),
            "wk": np.ascontiguousarray(Wk[sl, :].T).astype(np_dt),
            "wv": np.ascontiguousarray(Wv[sl, :].T).astype(np_dt),
            "bqk": np.ascontiguousarray(bqk, dtype=np.float32),
            "bv": np.ascontiguousarray(bv[None, sl]).astype(np_dt),
            "onesd": np.ones((128, 128), np_dt),
            "zerosd": np.zeros((64, S), np_dt),
        })
    return in_maps


def run(inputs, mode=MODE, trace=False):
    nc = _get_nc(mode)
    in_maps = _prep_inputs(inputs, mode)
    res = bass_utils.run_bass_kernel_spmd(
        nc, in_maps, core_ids=list(range(NCORES)), trace=trace)

    masks = np.asarray(inputs["masks"], np.float32)
    query = np.asarray(inputs["query"], np.float32)
    out = np.empty((B, S, H), np.float32)
    for c in range(NCORES):
        b, g = c // GROUPS, c % GROUPS
        hid = res.results[c]["hid"].reshape(HL, DH + 1, S)
        hT = hid[:, :DH, :]                      # (HL, DH, S)
        se = hid[:, DH, :]                       # (HL, S)
        blk = (hT / se[:, None, :]).transpose(2, 0, 1).reshape(S, GH)
        out[b, :, g * GH:(g + 1) * GH] = blk
    out = out * masks[:, :, None] + query
    return out, res


def kernel(**inputs) -> np.ndarray:
    out, _ = run(inputs)
    return out


# revision 18
# speedup vs baseline: 1.1836x; 1.0241x over previous
"""Multi-head attention (ReLU-gated projections) on 8 Trainium2 NeuronCores.

Problem (hardcoded): B=4, S=1024, H=1024, NH=16, DH=64.
  qp = relu(q @ Wq.T + bq); kp, vp likewise
  alpha = softmax(qh @ kh.T / sqrt(DH)) * mask[q]
  out = (alpha @ vh).reshape(B,S,H) + query

Sharding: 8 cores = 4 batches x 2 head-groups (8 heads / 512 hidden cols each).

Per-core device kernel (all in transposed "hidden-on-partitions" layout):
  stage 1: qpT[o,s], kpT[o,s] (transposed) and vp[s,o] (normal) projections
           with fused bias+relu. Contraction over h via PE; inputs fed
           host-pre-transposed (xT = x.T per batch).
  stage 2: per head: alphaT[k,q] = khT.T @ qhT (K=64); P=exp(alpha/8) on ACT
           (no max subtraction needed: alpha/8 <= ~5); AV via PE with a ones
           column appended to v so row 64 of the output accumulates
           sumexp[q] for free.  Output: unnormalized hidT (64,S) + sumexp (S)
           per head; host divides, applies mask, adds residual.
"""
import sys

sys.path.insert(0, "/opt/trn_rl_repo")

import os
import numpy as np
import ml_dtypes

import concourse.bass as bass
import concourse.tile as tile
from concourse import bacc, mybir
from concourse import bass_utils

if os.environ.get("BASS_LDW_OPT", "0") == "1":
    _orig_run_command = bass_utils.run_command

    def _patched_run_command(cmd, **kw):
        cmd = ["--enable-ldw-opt=true" if c == "--enable-ldw-opt=false" else c
               for c in cmd]
        return _orig_run_command(cmd, **kw)

    bass_utils.run_command = _patched_run_command

B, S, H = 4, 1024, 1024
NH, DH = 16, 64
NCORES = 8
GROUPS = 2          # head-groups (tensor-parallel dim)
HL = NH // GROUPS   # heads per core = 8
GH = H // GROUPS    # hidden cols per core = 512
KT = H // 128       # contraction k-tiles = 8
OT = GH // 128      # output o-tiles per core = 4
SCALE = 1.0 / float(np.sqrt(DH))

# matmul precision mode: "f32" (exact, 4 cyc/row), "f32r" (TF32-ish, 1 cyc/row),
# "bf16" (1 cyc/row, smallest footprint)
MODE = os.environ.get("BASS_MM_DT", "f32r")
ALPHA_ILV = os.environ.get("BASS_ALPHA_ILV", "1") == "1"

F32 = mybir.dt.float32
F32R = mybir.dt.float32r
BF16 = mybir.dt.bfloat16


def _cfg(mode):
    if mode == "bf16":
        return dict(np_dt=ml_dtypes.bfloat16, io_dt=BF16, st_dt=BF16,
                    cast=False, pt_bufs=36, hid_bufs=3, x_bufs=16,
                    shift_alphas=True, kz=True)
    if mode == "f32r":
        # float32r end-to-end: walrus requires f32r matmul inputs to be
        # *produced* as f32r (DMA loads + DVE/ACT evacuations), not bitcast.
        return dict(np_dt=np.float32, io_dt=F32R, st_dt=F32R,
                    cast=False, pt_bufs=9, hid_bufs=2, x_bufs=8,
                    shift_alphas=False, kz=False)
    return dict(np_dt=np.float32, io_dt=F32, st_dt=F32,
                cast=False, pt_bufs=9, hid_bufs=2, x_bufs=8,
                shift_alphas=False, kz=False)


def _mm(ap, cast):
    return ap.bitcast(F32R) if cast else ap


def build(mode):
    cfg = _cfg(mode)
    io_dt, st_dt, cast = cfg["io_dt"], cfg["st_dt"], cfg["cast"]
    nc = bacc.Bacc("TRN2", target_bir_lowering=False, debug=False,
                   num_devices=NCORES)

    xq_d = nc.dram_tensor("xq", [H, S], io_dt, kind="ExternalInput").ap()
    xk_d = nc.dram_tensor("xk", [H, S], io_dt, kind="ExternalInput").ap()
    xv_d = nc.dram_tensor("xv", [H, S], io_dt, kind="ExternalInput").ap()
    wq_d = nc.dram_tensor("wq", [H, GH], io_dt, kind="ExternalInput").ap()
    wk_d = nc.dram_tensor("wk", [H, GH], io_dt, kind="ExternalInput").ap()
    wv_d = nc.dram_tensor("wv", [H, GH], io_dt, kind="ExternalInput").ap()
    bqk_d = nc.dram_tensor("bqk", [128, 2 * OT], F32, kind="ExternalInput").ap()
    bv_d = nc.dram_tensor("bv", [1, GH], io_dt, kind="ExternalInput").ap()
    ones_d = nc.dram_tensor("onesd", [128, 128], io_dt,
                            kind="ExternalInput").ap()
    zeros_d = nc.dram_tensor("zerosd", [64, S], io_dt,
                             kind="ExternalInput").ap()
    hid_d = nc.dram_tensor("hid", [HL * (DH + 1), S], F32,
                           kind="ExternalOutput").ap()

    with tile.TileContext(nc) as tc:
        with tc.tile_pool(name="sb", bufs=1) as sb, \
             tc.tile_pool(name="ps", bufs=1, space="PSUM") as ps:

            full_x = mode == "bf16"   # x resident for full S vs per-chunk

            # ---- persistent tiles; one big DMA per tensor (>=1MB, descriptor
            #      runs of 1-2KB/partition), spread across the three DGE rings
            #      (sync / scalar / gpsimd) so loads overlap ----
            wq_t = sb.tile([128, KT * GH], io_dt, tag="wq", name="wq")
            wk_t = sb.tile([128, KT * GH], io_dt, tag="wk", name="wk")
            wv_t = sb.tile([128, KT * GH], io_dt, tag="wv", name="wv")
            qp_t = [sb.tile([128, S], st_dt, tag=f"qp{t}", name=f"qp{t}")
                    for t in range(OT)]
            KZ = cfg["kz"]
            if KZ:
                # zero-padded K copies: kz[t][h] holds head h's kh rows in its
                # own 64-partition half, zeros in the other -> full-K=128
                # alpha matmuls whose weight loads pipeline like any other MM
                kz_t = [[sb.tile([128, S], st_dt, tag=f"kz{t}{h}",
                                 name=f"kz{t}{h}") for h in range(2)]
                        for t in range(OT)]
                for t in range(OT):
                    nc.vector.memset(kz_t[t][0][64:128, :], 0.0)
                    nc.vector.memset(kz_t[t][1][0:64, :], 0.0)
            else:
                kp_t = [sb.tile([128, S], st_dt, tag=f"kp{t}",
                                name=f"kp{t}") for t in range(OT)]
            # v laid out [k-tile x head x (64 v cols + ones col)]
            VW = HL * (DH + 1)
            vp_t = sb.tile([128, KT * VW], st_dt, tag="vp", name="vp")
            bqk_t = sb.tile([128, 2 * OT], F32, tag="bqk", name="bqk")
            bv_t = sb.tile([1, GH], io_dt, tag="bv", name="bv")
            ones_t = sb.tile([1, 128], io_dt, tag="ones", name="ones")

            def w_ld(w_t, w_d, eng):
                # SBUF [128, k*GH + o]  <-  DRAM [(k p) o]
                eng.dma_start(w_t[:].rearrange("p (k o) -> p k o", o=GH),
                              w_d.rearrange("(k p) o -> p k o", p=128))

            x_t = {}

            def x_ld(which, x_d, eng, sc):
                # one chunk (512 s-cols) of one input, laid [128, k*512+s]
                tag = f"x{which}{sc}" if full_x else f"x{which}"
                t = sb.tile([128, KT * 512], io_dt, tag=tag,
                            name=f"x{which}_{sc}")
                eng.dma_start(
                    t[:].rearrange("p (k s) -> p k s", s=512),
                    x_d.rearrange("(k p) s -> p k s",
                                  p=128)[:, :, sc * 512:(sc + 1) * 512])
                x_t[(which, sc)] = t

            def xap(which, sc, k):
                return x_t[(which, sc)][:, k * 512:(k + 1) * 512]

            nc.sync.dma_start(bv_t[:], bv_d)
            nc.sync.dma_start(ones_t[:], ones_d[0:1, :])
            nc.sync.dma_start(bqk_t[:], bqk_d)
            ones64_t = sb.tile([128, KT * HL], io_dt, tag="ones64",
                               name="ones64")
            nc.sync.dma_start(ones64_t[:], ones_d[:, 0:KT * HL])
            x_ld("q", xq_d, nc.scalar, 0)
            w_ld(wq_t, wq_d, nc.sync)
            x_ld("k", xk_d, nc.sync, 0)
            w_ld(wk_t, wk_d, nc.scalar)
            x_ld("v", xv_d, nc.scalar, 0)
            w_ld(wv_t, wv_d, nc.sync)

            # HAM warmup: ~7us of tiny matmuls on early-arriving const tiles
            # so the real matmuls start at 2.4GHz instead of 1.2
            warm = ps.tile([128, 512], F32, tag="av", bufs=2, name="warm")
            for i in range(20):
                nc.tensor.matmul(warm[:], _mm(ones_t[:], cast),
                                 _mm(bv_t[:], cast), start=True, stop=True)
            if full_x:
                x_ld("q", xq_d, nc.sync, 1)
                x_ld("k", xk_d, nc.sync, 1)
                x_ld("v", xv_d, nc.gpsimd, 1)
            v4 = vp_t[:].rearrange("p (k n c) -> p k n c", n=HL, c=DH + 1)
            nc.vector.tensor_copy(
                v4[:, :, :, DH:DH + 1],
                ones64_t[:].rearrange("p (k n one) -> p k n one", n=HL, one=1))

            def proj_qk(sc, ot):
                """one o-tile, one s-chunk of the transposed q/k projections"""
                for which, w_t, xw in (("q", wq_t, "q"), ("k", wk_t, "k")):
                    pp = ps.tile([128, 1024], F32, tag="alpha", bufs=3,
                                 name=f"pp{which}_{sc}_{ot}")
                    for k in range(KT):
                        nc.tensor.matmul(
                            pp[:, 0:512],
                            _mm(w_t[:, k * GH + ot * 128:
                                    k * GH + (ot + 1) * 128], cast),
                            _mm(xap(xw, sc, k), cast),
                            start=(k == 0), stop=(k == KT - 1))
                    wi = 0 if which == "q" else 1
                    bias = bqk_t[:, wi * OT + ot:wi * OT + ot + 1]
                    ssl = slice(sc * 512, (sc + 1) * 512)
                    if which == "q":
                        nc.vector.tensor_scalar(
                            qp_t[ot][:, ssl], pp[:, 0:512], bias, 0.0,
                            mybir.AluOpType.add, mybir.AluOpType.max)
                    elif KZ:
                        for h in range(2):
                            pr = slice(h * 64, h * 64 + 64)
                            nc.vector.tensor_scalar(
                                kz_t[ot][h][pr, ssl], pp[pr, 0:512],
                                bias[pr, :], 0.0,
                                mybir.AluOpType.add, mybir.AluOpType.max)
                    else:
                        nc.vector.tensor_scalar(
                            kp_t[ot][:, ssl], pp[:, 0:512], bias, 0.0,
                            mybir.AluOpType.add, mybir.AluOpType.max)

            def proj_v(sc, j):
                """one s-tile (128 rows of vp) within chunk sc"""
                st = sc * 4 + j
                pp = ps.tile([128, 1024], F32, tag="alpha", bufs=3,
                             name=f"ppv_{st}")
                nc.tensor.matmul(pp[:, 0:512], _mm(ones_t[:], cast),
                                 _mm(bv_t[:], cast), start=True, stop=False)
                for k in range(KT):
                    nc.tensor.matmul(
                        pp[:, 0:512],
                        _mm(xap("v", sc, k)[:, j * 128:(j + 1) * 128], cast),
                        _mm(wv_t[:, k * GH:(k + 1) * GH], cast),
                        start=False, stop=(k == KT - 1))
                v3 = vp_t[:, st * VW:(st + 1) * VW].rearrange(
                    "p (n c) -> p n c", c=DH + 1)
                p3 = pp[:, 0:512].rearrange("p (n c) -> p n c", c=DH)
                nc.vector.tensor_scalar(
                    v3[:, :, 0:DH], p3, 0.0, None, mybir.AluOpType.max)

            pt_all = {}

            def alphas(n0):
                """alpha + exp for head pair (n0, n0+1); the two heads live on
                disjoint 64-partition halves of o-tile n0//2, so adjacent
                matmuls target disjoint PE row-groups and overlap."""
                t = n0 // 2
                pts0, pts1 = [], []
                for k in range(KT):
                    apts = []
                    for h in range(2):
                        apt = ps.tile([128, 1024], F32, tag="alpha", bufs=3,
                                      name=f"alp_{n0 + h}_{k}")
                        apts.append(apt)
                    for qc in range(2):
                        for h in range(2):
                            nc.tensor.matmul(
                                apts[h][:, qc * 512:(qc + 1) * 512],
                                _mm(kz_t[t][h][:, k * 128:(k + 1) * 128],
                                    cast),
                                _mm(qp_t[t][:, qc * 512:(qc + 1) * 512],
                                    cast),
                                start=True, stop=True)
                    for h, pts in ((0, pts0), (1, pts1)):
                        pt = sb.tile([128, 1024], st_dt, tag="pt",
                                     bufs=cfg["pt_bufs"], name=f"pt_{n0 + h}_{k}")
                        nc.scalar.activation(pt[:], apts[h][:],
                                             mybir.ActivationFunctionType.Exp,
                                             scale=SCALE)
                        pts.append(pt)
                pt_all[n0] = pts0
                pt_all[n0 + 1] = pts1

            def head_seq(n):
                """unpaired alpha+exp then AV for one head (low pt_bufs modes)"""
                t, off = n // 2, (n % 2) * 64
                pts = []
                for k in range(KT):
                    apt = ps.tile([128, 1024], F32, tag="alpha", bufs=3,
                                  name=f"alp_{n}_{k}")
                    for qc in range(2):
                        nc.tensor.matmul(
                            apt[:, qc * 512:(qc + 1) * 512],
                            _mm(kp_t[t][off:off + 64,
                                        k * 128:(k + 1) * 128], cast),
                            _mm(qp_t[t][off:off + 64,
                                        qc * 512:(qc + 1) * 512], cast),
                            start=True, stop=True)
                    pt = sb.tile([128, 1024], st_dt, tag="pt",
                                 bufs=cfg["pt_bufs"], name=f"pt_{n}_{k}")
                    nc.scalar.activation(pt[:], apt[:],
                                         mybir.ActivationFunctionType.Exp,
                                         scale=SCALE)
                    pts.append(pt)
                pt_all[n] = pts

            def avs(n):
                pts = pt_all.pop(n)
                hid_t = sb.tile([DH + 1, S], F32, tag="hid",
                                bufs=cfg["hid_bufs"], name=f"hid_{n}")
                for qc in range(2):
                    av = ps.tile([DH + 1, 512], F32, tag="av", bufs=2,
                                 name=f"av_{n}_{qc}")
                    for k in range(KT):
                        nc.tensor.matmul(
                            av[:],
                            _mm(vp_t[:, k * VW + n * (DH + 1):
                                     k * VW + (n + 1) * (DH + 1)], cast),
                            _mm(pts[k][:, qc * 512:(qc + 1) * 512], cast),
                            start=(k == 0), stop=(k == KT - 1))
                    nc.vector.tensor_copy(
                        hid_t[:, qc * 512:(qc + 1) * 512], av[:])
                    nc.sync.dma_start(
                        hid_d[n * (DH + 1):(n + 1) * (DH + 1),
                              qc * 512:(qc + 1) * 512],
                        hid_t[:, qc * 512:(qc + 1) * 512])

            # ---- emission schedule ----
            for ot in range(OT):
                proj_qk(0, ot)
            for j in range(4):
                proj_v(0, j)
            if not full_x:
                x_ld("q", xq_d, nc.sync, 1)
                x_ld("k", xk_d, nc.sync, 1)
                x_ld("v", xv_d, nc.gpsimd, 1)
            if cfg["shift_alphas"]:
                proj_qk(1, 0)
                alphas(0)
                for j in range(4):
                    proj_v(1, j)
                proj_qk(1, 1)
                alphas(2)
                avs(0)
                avs(1)
                proj_qk(1, 2)
                alphas(4)
                avs(2)
                avs(3)
                proj_qk(1, 3)
                alphas(6)
                avs(4)
                avs(5)
                avs(6)
                avs(7)
            else:
                proj_qk(1, 0)
                head_seq(0)
                for j in range(4):
                    proj_v(1, j)
                head_seq(1)
                avs(0)
                avs(1)
                for ot in range(1, OT):
                    proj_qk(1, ot)
                    head_seq(2 * ot)
                    avs(2 * ot)
                    head_seq(2 * ot + 1)
                    avs(2 * ot + 1)

    nc.compile()
    return nc


_NC_CACHE = {}


def _get_nc(mode):
    if mode not in _NC_CACHE:
        _NC_CACHE[mode] = build(mode)
    return _NC_CACHE[mode]


def _prep_inputs(inputs, mode):
    cfg = _cfg(mode)
    np_dt = cfg["np_dt"]
    q = np.asarray(inputs["query"], np.float32)
    k = np.asarray(inputs["key"], np.float32)
    v = np.asarray(inputs["value"], np.float32)
    Wq = np.asarray(inputs["Wq"], np.float32)
    Wk = np.asarray(inputs["Wk"], np.float32)
    Wv = np.asarray(inputs["Wv"], np.float32)
    bq = np.asarray(inputs["bq"], np.float32)
    bk = np.asarray(inputs["bk"], np.float32)
    bv = np.asarray(inputs["bv"], np.float32)

    xq = [np.ascontiguousarray(q[b].T).astype(np_dt) for b in range(B)]
    xk = [np.ascontiguousarray(k[b].T).astype(np_dt) for b in range(B)]
    xv = [np.ascontiguousarray(v[b].T).astype(np_dt) for b in range(B)]
    in_maps = []
    for c in range(NCORES):
        b, g = c // GROUPS, c % GROUPS
        sl = slice(g * GH, (g + 1) * GH)
        bqk = np.stack([bq[sl].reshape(OT, 128).T,
                        bk[sl].reshape(OT, 128).T], 1).reshape(128, 2 * OT)
        in_maps.append({
            "xq": xq[b], "xk": xk[b], "xv": xv[b],
            "wq": np.ascontiguousarray(Wq[sl, :].T).astype(np_dt),
            "wk": np.ascontiguousarray(Wk[sl, :].T).astype(np_dt),
            "wv": np.ascontiguousarray(Wv[sl, :].T).astype(np_dt),
            "bqk": np.ascontiguousarray(bqk, dtype=np.float32),
            "bv": np.ascontiguousarray(bv[None, sl]).astype(np_dt),
            "onesd": np.ones((128, 128), np_dt),
            "zerosd": np.zeros((64, S), np_dt),
        })
    return in_maps


def run(inputs, mode=MODE, trace=False):
    nc = _get_nc(mode)
    in_maps = _prep_inputs(inputs, mode)
    res = bass_utils.run_bass_kernel_spmd(
        nc, in_maps, core_ids=list(range(NCORES)), trace=trace)

    masks = np.asarray(inputs["masks"], np.float32)
    query = np.asarray(inputs["query"], np.float32)
    out = np.empty((B, S, H), np.float32)
    for c in range(NCORES):
        b, g = c // GROUPS, c % GROUPS
        hid = res.results[c]["hid"].reshape(HL, DH + 1, S)
        hT = hid[:, :DH, :]                      # (HL, DH, S)
        se = hid[:, DH, :]                       # (HL, S)
        blk = (hT / se[:, None, :]).transpose(2, 0, 1).reshape(S, GH)
        out[b, :, g * GH:(g + 1) * GH] = blk
    out = out * masks[:, :, None] + query
    return out, res


def kernel(**inputs) -> np.ndarray:
    out, _ = run(inputs)
    return out


# revision 19
# speedup vs baseline: 1.1858x; 1.0019x over previous
"""Multi-head attention (ReLU-gated projections) on 8 Trainium2 NeuronCores.

Problem (hardcoded): B=4, S=1024, H=1024, NH=16, DH=64.
  qp = relu(q @ Wq.T + bq); kp, vp likewise
  alpha = softmax(qh @ kh.T / sqrt(DH)) * mask[q]
  out = (alpha @ vh).reshape(B,S,H) + query

Sharding: 8 cores = 4 batches x 2 head-groups (8 heads / 512 hidden cols each).

Per-core device kernel (all in transposed "hidden-on-partitions" layout):
  stage 1: qpT[o,s], kpT[o,s] (transposed) and vp[s,o] (normal) projections
           with fused bias+relu. Contraction over h via PE; inputs fed
           host-pre-transposed (xT = x.T per batch).
  stage 2: per head: alphaT[k,q] = khT.T @ qhT (K=64); P=exp(alpha/8) on ACT
           (no max subtraction needed: alpha/8 <= ~5); AV via PE with a ones
           column appended to v so row 64 of the output accumulates
           sumexp[q] for free.  Output: unnormalized hidT (64,S) + sumexp (S)
           per head; host divides, applies mask, adds residual.
"""
import sys

sys.path.insert(0, "/opt/trn_rl_repo")

import os
import numpy as np
import ml_dtypes

import concourse.bass as bass
import concourse.tile as tile
from concourse import bacc, mybir
from concourse import bass_utils

if os.environ.get("BASS_LDW_OPT", "0") == "1":
    _orig_run_command = bass_utils.run_command

    def _patched_run_command(cmd, **kw):
        cmd = ["--enable-ldw-opt=true" if c == "--enable-ldw-opt=false" else c
               for c in cmd]
        return _orig_run_command(cmd, **kw)

    bass_utils.run_command = _patched_run_command

B, S, H = 4, 1024, 1024
NH, DH = 16, 64
NCORES = 8
GROUPS = 2          # head-groups (tensor-parallel dim)
HL = NH // GROUPS   # heads per core = 8
GH = H // GROUPS    # hidden cols per core = 512
KT = H // 128       # contraction k-tiles = 8
OT = GH // 128      # output o-tiles per core = 4
SCALE = 1.0 / float(np.sqrt(DH))

# matmul precision mode: "f32" (exact, 4 cyc/row), "f32r" (TF32-ish, 1 cyc/row),
# "bf16" (1 cyc/row, smallest footprint)
MODE = os.environ.get("BASS_MM_DT", "f32r")
ALPHA_ILV = os.environ.get("BASS_ALPHA_ILV", "1") == "1"

F32 = mybir.dt.float32
F32R = mybir.dt.float32r
BF16 = mybir.dt.bfloat16


def _cfg(mode):
    if mode == "bf16":
        return dict(np_dt=ml_dtypes.bfloat16, io_dt=BF16, st_dt=BF16,
                    cast=False, pt_bufs=36, hid_bufs=3, x_bufs=16,
                    shift_alphas=True, kz=True)
    if mode == "f32r":
        # float32r end-to-end: walrus requires f32r matmul inputs to be
        # *produced* as f32r (DMA loads + DVE/ACT evacuations), not bitcast.
        return dict(np_dt=np.float32, io_dt=F32R, st_dt=F32R,
                    cast=False, pt_bufs=9, hid_bufs=2, x_bufs=8,
                    shift_alphas=False, kz=False)
    return dict(np_dt=np.float32, io_dt=F32, st_dt=F32,
                cast=False, pt_bufs=9, hid_bufs=2, x_bufs=8,
                shift_alphas=False, kz=False)


def _mm(ap, cast):
    return ap.bitcast(F32R) if cast else ap


def build(mode):
    cfg = _cfg(mode)
    io_dt, st_dt, cast = cfg["io_dt"], cfg["st_dt"], cfg["cast"]
    nc = bacc.Bacc("TRN2", target_bir_lowering=False, debug=False,
                   num_devices=NCORES)

    xq_d = nc.dram_tensor("xq", [H, S], io_dt, kind="ExternalInput").ap()
    xk_d = nc.dram_tensor("xk", [H, S], io_dt, kind="ExternalInput").ap()
    xv_d = nc.dram_tensor("xv", [H, S], io_dt, kind="ExternalInput").ap()
    wq_d = nc.dram_tensor("wq", [H, GH], io_dt, kind="ExternalInput").ap()
    wk_d = nc.dram_tensor("wk", [H, GH], io_dt, kind="ExternalInput").ap()
    wv_d = nc.dram_tensor("wv", [H, GH], io_dt, kind="ExternalInput").ap()
    bqk_d = nc.dram_tensor("bqk", [128, 2 * OT], F32, kind="ExternalInput").ap()
    bv_d = nc.dram_tensor("bv", [1, GH], io_dt, kind="ExternalInput").ap()
    ones_d = nc.dram_tensor("onesd", [128, 128], io_dt,
                            kind="ExternalInput").ap()
    zeros_d = nc.dram_tensor("zerosd", [64, S], io_dt,
                             kind="ExternalInput").ap()
    hid_d = nc.dram_tensor("hid", [HL * (DH + 1), S], F32,
                           kind="ExternalOutput").ap()

    with tile.TileContext(nc) as tc:
        with tc.tile_pool(name="sb", bufs=1) as sb, \
             tc.tile_pool(name="ps", bufs=1, space="PSUM") as ps:

            full_x = mode == "bf16"   # x resident for full S vs per-chunk

            # ---- persistent tiles; one big DMA per tensor (>=1MB, descriptor
            #      runs of 1-2KB/partition), spread across the three DGE rings
            #      (sync / scalar / gpsimd) so loads overlap ----
            wq_t = sb.tile([128, KT * GH], io_dt, tag="wq", name="wq")
            wk_t = sb.tile([128, KT * GH], io_dt, tag="wk", name="wk")
            wv_t = sb.tile([128, KT * GH], io_dt, tag="wv", name="wv")
            qp_t = [sb.tile([128, S], st_dt, tag=f"qp{t}", name=f"qp{t}")
                    for t in range(OT)]
            KZ = cfg["kz"]
            if KZ:
                # zero-padded K copies: kz[t][h] holds head h's kh rows in its
                # own 64-partition half, zeros in the other -> full-K=128
                # alpha matmuls whose weight loads pipeline like any other MM
                kz_t = [[sb.tile([128, S], st_dt, tag=f"kz{t}{h}",
                                 name=f"kz{t}{h}") for h in range(2)]
                        for t in range(OT)]
                for t in range(OT):
                    nc.vector.memset(kz_t[t][0][64:128, :], 0.0)
                    nc.vector.memset(kz_t[t][1][0:64, :], 0.0)
            else:
                kp_t = [sb.tile([128, S], st_dt, tag=f"kp{t}",
                                name=f"kp{t}") for t in range(OT)]
            # v laid out [k-tile x head x (64 v cols + ones col)]
            VW = HL * (DH + 1)
            vp_t = sb.tile([128, KT * VW], st_dt, tag="vp", name="vp")
            bqk_t = sb.tile([128, 2 * OT], F32, tag="bqk", name="bqk")
            bv_t = sb.tile([1, GH], io_dt, tag="bv", name="bv")
            ones_t = sb.tile([1, 128], io_dt, tag="ones", name="ones")

            def w_ld(w_t, w_d, eng):
                # SBUF [128, k*GH + o]  <-  DRAM [(k p) o]
                eng.dma_start(w_t[:].rearrange("p (k o) -> p k o", o=GH),
                              w_d.rearrange("(k p) o -> p k o", p=128))

            x_t = {}

            def x_ld(which, x_d, eng, sc, eng2=None):
                # one chunk (512 s-cols) of one input, laid [128, k*512+s];
                # optionally split across two DGE rings for latency
                tag = f"x{which}{sc}" if full_x else f"x{which}"
                t = sb.tile([128, KT * 512], io_dt, tag=tag,
                            name=f"x{which}_{sc}")
                src3 = x_d.rearrange("(k p) s -> p k s",
                                     p=128)[:, :, sc * 512:(sc + 1) * 512]
                dst3 = t[:].rearrange("p (k s) -> p k s", s=512)
                if eng2 is None:
                    eng.dma_start(dst3, src3)
                else:
                    h = KT // 2
                    eng.dma_start(dst3[:, :h], src3[:, :h])
                    eng2.dma_start(dst3[:, h:], src3[:, h:])
                x_t[(which, sc)] = t

            def xap(which, sc, k):
                return x_t[(which, sc)][:, k * 512:(k + 1) * 512]

            nc.sync.dma_start(bv_t[:], bv_d)
            nc.sync.dma_start(ones_t[:], ones_d[0:1, :])
            nc.sync.dma_start(bqk_t[:], bqk_d)
            ones64_t = sb.tile([128, KT * HL], io_dt, tag="ones64",
                               name="ones64")
            nc.sync.dma_start(ones64_t[:], ones_d[:, 0:KT * HL])
            x_ld("q", xq_d, nc.scalar, 0)
            w_ld(wq_t, wq_d, nc.sync)
            x_ld("k", xk_d, nc.sync, 0)
            w_ld(wk_t, wk_d, nc.scalar)
            x_ld("v", xv_d, nc.scalar, 0)
            w_ld(wv_t, wv_d, nc.sync)

            # HAM warmup: ~7us of tiny matmuls on early-arriving const tiles
            # so the real matmuls start at 2.4GHz instead of 1.2
            warm = ps.tile([128, 512], F32, tag="av", bufs=2, name="warm")
            for i in range(20):
                nc.tensor.matmul(warm[:], _mm(ones_t[:], cast),
                                 _mm(bv_t[:], cast), start=True, stop=True)
            if full_x:
                x_ld("q", xq_d, nc.sync, 1)
                x_ld("k", xk_d, nc.sync, 1)
                x_ld("v", xv_d, nc.gpsimd, 1)
            v4 = vp_t[:].rearrange("p (k n c) -> p k n c", n=HL, c=DH + 1)
            nc.vector.tensor_copy(
                v4[:, :, :, DH:DH + 1],
                ones64_t[:].rearrange("p (k n one) -> p k n one", n=HL, one=1))

            def proj_qk(sc, ot, only=None):
                """one o-tile, one s-chunk of the transposed q/k projections"""
                for which, w_t, xw in (("q", wq_t, "q"), ("k", wk_t, "k")):
                    if only is not None and which != only:
                        continue
                    pp = ps.tile([128, 1024], F32, tag="alpha", bufs=3,
                                 name=f"pp{which}_{sc}_{ot}")
                    for k in range(KT):
                        nc.tensor.matmul(
                            pp[:, 0:512],
                            _mm(w_t[:, k * GH + ot * 128:
                                    k * GH + (ot + 1) * 128], cast),
                            _mm(xap(xw, sc, k), cast),
                            start=(k == 0), stop=(k == KT - 1))
                    wi = 0 if which == "q" else 1
                    bias = bqk_t[:, wi * OT + ot:wi * OT + ot + 1]
                    ssl = slice(sc * 512, (sc + 1) * 512)
                    if which == "q":
                        nc.vector.tensor_scalar(
                            qp_t[ot][:, ssl], pp[:, 0:512], bias, 0.0,
                            mybir.AluOpType.add, mybir.AluOpType.max)
                    elif KZ:
                        for h in range(2):
                            pr = slice(h * 64, h * 64 + 64)
                            nc.vector.tensor_scalar(
                                kz_t[ot][h][pr, ssl], pp[pr, 0:512],
                                bias[pr, :], 0.0,
                                mybir.AluOpType.add, mybir.AluOpType.max)
                    else:
                        nc.vector.tensor_scalar(
                            kp_t[ot][:, ssl], pp[:, 0:512], bias, 0.0,
                            mybir.AluOpType.add, mybir.AluOpType.max)

            def proj_v(sc, j):
                """one s-tile (128 rows of vp) within chunk sc"""
                st = sc * 4 + j
                pp = ps.tile([128, 1024], F32, tag="alpha", bufs=3,
                             name=f"ppv_{st}")
                nc.tensor.matmul(pp[:, 0:512], _mm(ones_t[:], cast),
                                 _mm(bv_t[:], cast), start=True, stop=False)
                for k in range(KT):
                    nc.tensor.matmul(
                        pp[:, 0:512],
                        _mm(xap("v", sc, k)[:, j * 128:(j + 1) * 128], cast),
                        _mm(wv_t[:, k * GH:(k + 1) * GH], cast),
                        start=False, stop=(k == KT - 1))
                v3 = vp_t[:, st * VW:(st + 1) * VW].rearrange(
                    "p (n c) -> p n c", c=DH + 1)
                p3 = pp[:, 0:512].rearrange("p (n c) -> p n c", c=DH)
                nc.vector.tensor_scalar(
                    v3[:, :, 0:DH], p3, 0.0, None, mybir.AluOpType.max)

            pt_all = {}

            def alphas(n0):
                """alpha + exp for head pair (n0, n0+1); the two heads live on
                disjoint 64-partition halves of o-tile n0//2, so adjacent
                matmuls target disjoint PE row-groups and overlap."""
                t = n0 // 2
                pts0, pts1 = [], []
                for k in range(KT):
                    apts = []
                    for h in range(2):
                        apt = ps.tile([128, 1024], F32, tag="alpha", bufs=3,
                                      name=f"alp_{n0 + h}_{k}")
                        apts.append(apt)
                    for qc in range(2):
                        for h in range(2):
                            nc.tensor.matmul(
                                apts[h][:, qc * 512:(qc + 1) * 512],
                                _mm(kz_t[t][h][:, k * 128:(k + 1) * 128],
                                    cast),
                                _mm(qp_t[t][:, qc * 512:(qc + 1) * 512],
                                    cast),
                                start=True, stop=True)
                    for h, pts in ((0, pts0), (1, pts1)):
                        pt = sb.tile([128, 1024], st_dt, tag="pt",
                                     bufs=cfg["pt_bufs"], name=f"pt_{n0 + h}_{k}")
                        nc.scalar.activation(pt[:], apts[h][:],
                                             mybir.ActivationFunctionType.Exp,
                                             scale=SCALE)
                        pts.append(pt)
                pt_all[n0] = pts0
                pt_all[n0 + 1] = pts1

            def head_seq(n):
                """unpaired alpha+exp then AV for one head (low pt_bufs modes)"""
                t, off = n // 2, (n % 2) * 64
                pts = []
                for k in range(KT):
                    apt = ps.tile([128, 1024], F32, tag="alpha", bufs=3,
                                  name=f"alp_{n}_{k}")
                    for qc in range(2):
                        nc.tensor.matmul(
                            apt[:, qc * 512:(qc + 1) * 512],
                            _mm(kp_t[t][off:off + 64,
                                        k * 128:(k + 1) * 128], cast),
                            _mm(qp_t[t][off:off + 64,
                                        qc * 512:(qc + 1) * 512], cast),
                            start=True, stop=True)
                    pt = sb.tile([128, 1024], st_dt, tag="pt",
                                 bufs=cfg["pt_bufs"], name=f"pt_{n}_{k}")
                    nc.scalar.activation(pt[:], apt[:],
                                         mybir.ActivationFunctionType.Exp,
                                         scale=SCALE)
                    pts.append(pt)
                pt_all[n] = pts

            def avs(n):
                pts = pt_all.pop(n)
                hid_t = sb.tile([DH + 1, S], F32, tag="hid",
                                bufs=cfg["hid_bufs"], name=f"hid_{n}")
                for qc in range(2):
                    av = ps.tile([DH + 1, 512], F32, tag="av", bufs=2,
                                 name=f"av_{n}_{qc}")
                    for k in range(KT):
                        nc.tensor.matmul(
                            av[:],
                            _mm(vp_t[:, k * VW + n * (DH + 1):
                                     k * VW + (n + 1) * (DH + 1)], cast),
                            _mm(pts[k][:, qc * 512:(qc + 1) * 512], cast),
                            start=(k == 0), stop=(k == KT - 1))
                    nc.vector.tensor_copy(
                        hid_t[:, qc * 512:(qc + 1) * 512], av[:])
                    nc.sync.dma_start(
                        hid_d[n * (DH + 1):(n + 1) * (DH + 1),
                              qc * 512:(qc + 1) * 512],
                        hid_t[:, qc * 512:(qc + 1) * 512])

            # ---- emission schedule ----
            if cfg["shift_alphas"]:
                for ot in range(OT):
                    proj_qk(0, ot, only="q")
                for ot in range(OT):
                    proj_qk(0, ot, only="k")
            else:
                for ot in range(OT):
                    proj_qk(0, ot)
            for j in range(4):
                proj_v(0, j)
            if not full_x:
                x_ld("q", xq_d, nc.sync, 1)
                x_ld("k", xk_d, nc.sync, 1)
                x_ld("v", xv_d, nc.gpsimd, 1)
            if cfg["shift_alphas"]:
                proj_qk(1, 0)
                alphas(0)
                for j in range(4):
                    proj_v(1, j)
                proj_qk(1, 1)
                alphas(2)
                avs(0)
                avs(1)
                proj_qk(1, 2)
                alphas(4)
                avs(2)
                avs(3)
                proj_qk(1, 3)
                alphas(6)
                avs(4)
                avs(5)
                avs(6)
                avs(7)
            else:
                proj_qk(1, 0)
                head_seq(0)
                for j in range(4):
                    proj_v(1, j)
                head_seq(1)
                avs(0)
                avs(1)
                for ot in range(1, OT):
                    proj_qk(1, ot)
                    head_seq(2 * ot)
                    avs(2 * ot)
                    head_seq(2 * ot + 1)
                    avs(2 * ot + 1)

    nc.compile()
    return nc


_NC_CACHE = {}


def _get_nc(mode):
    if mode not in _NC_CACHE:
        _NC_CACHE[mode] = build(mode)
    return _NC_CACHE[mode]


def _prep_inputs(inputs, mode):
    cfg = _cfg(mode)
    np_dt = cfg["np_dt"]
    q = np.asarray(inputs["query"], np.float32)
    k = np.asarray(inputs["key"], np.float32)
    v = np.asarray(inputs["value"], np.float32)
    Wq = np.asarray(inputs["Wq"], np.float32)
    Wk = np.asarray(inputs["Wk"], np.float32)
    Wv = np.asarray(inputs["Wv"], np.float32)
    bq = np.asarray(inputs["bq"], np.float32)
    bk = np.asarray(inputs["bk"], np.float32)
    bv = np.asarray(inputs["bv"], np.float32)

    xq = [np.ascontiguousarray(q[b].T).astype(np_dt) for b in range(B)]
    xk = [np.ascontiguousarray(k[b].T).astype(np_dt) for b in range(B)]
    xv = [np.ascontiguousarray(v[b].T).astype(np_dt) for b in range(B)]
    in_maps = []
    for c in range(NCORES):
        b, g = c // GROUPS, c % GROUPS
        sl = slice(g * GH, (g + 1) * GH)
        bqk = np.stack([bq[sl].reshape(OT, 128).T,
                        bk[sl].reshape(OT, 128).T], 1).reshape(128, 2 * OT)
        in_maps.append({
            "xq": xq[b], "xk": xk[b], "xv": xv[b],
            "wq": np.ascontiguousarray(Wq[sl, :].T).astype(np_dt),
            "wk": np.ascontiguousarray(Wk[sl, :].T).astype(np_dt),
            "wv": np.ascontiguousarray(Wv[sl, :].T).astype(np_dt),
            "bqk": np.ascontiguousarray(bqk, dtype=np.float32),
            "bv": np.ascontiguousarray(bv[None, sl]).astype(np_dt),
            "onesd": np.ones((128, 128), np_dt),
            "zerosd": np.zeros((64, S), np_dt),
        })
    return in_maps


def run(inputs, mode=MODE, trace=False):
    nc = _get_nc(mode)
    in_maps = _prep_inputs(inputs, mode)
    res = bass_utils.run_bass_kernel_spmd(
        nc, in_maps, core_ids=list(range(NCORES)), trace=trace)

    masks = np.asarray(inputs["masks"], np.float32)
    query = np.asarray(inputs["query"], np.float32)
    out = np.empty((B, S, H), np.float32)
    for c in range(NCORES):
        b, g = c // GROUPS, c % GROUPS
        hid = res.results[c]["hid"].reshape(HL, DH + 1, S)
        hT = hid[:, :DH, :]                      # (HL, DH, S)
        se = hid[:, DH, :]                       # (HL, S)
        blk = (hT / se[:, None, :]).transpose(2, 0, 1).reshape(S, GH)
        out[b, :, g * GH:(g + 1) * GH] = blk
    out = out * masks[:, :, None] + query
    return out, res


def kernel(**inputs) -> np.ndarray:
    out, _ = run(inputs)
    return out
